# revision 30
# baseline (speedup 1.0000x reference)
"""DualMotionGAN forward on 8 Trainium2 NeuronCores (Bass/Tile, float32r matmuls).

Distribution: encoder data-parallel over the 16 frames (2 per core, stride-2
convs phase-decomposed into full-K tap matmuls); 3-layer ConvLSTM 8-way
gate-channel split (each core computes 64 channels of each gate for all
samples; h is AllGathered every step); an AllToAll then hands each core the
(sample, row-half) slab of out_me it needs, so the two deconv generators,
the bilinear warp (elementwise: flow=tanh in (-1,1) touches only the 3x3
neighborhood), and the fuse conv run spatially split with no further
communication. All per-core variation enters via host-sliced inputs --
the device program is identical on every core.
"""
import os
import numpy as np

import concourse.bass as bass
from concourse import bacc
import concourse.mybir as mybir
import concourse.tile as tile

f32r = mybir.dt.float32r
f32 = mybir.dt.float32
AF = mybir.ActivationFunctionType
OP = mybir.AluOpType
P = 128
NCORES = 8
B, C, T, H, W = 4, 3, 4, 256, 256

# deconv tap table: (phase, kernel index, input shift)
DEC_YT = [(0, 1, 0), (1, 0, 0), (1, 2, 1)]
DEC_TAPS = [(py, px, ky, kx, dy, dx)
            for (py, ky, dy) in DEC_YT for (px, kx, dx) in DEC_YT]

STAGE = os.environ.get("KSTAGE", "full")
KN_LAYERS = int(os.environ.get("KN_LAYERS", "3"))
KN_STEPS = int(os.environ.get("KN_STEPS", "4"))


def row_tiles(nrows, width, maxn=512):
    rpt = max(1, maxn // width)
    out = []
    r = 0
    while r < nrows:
        n = min(rpt, nrows - r)
        out.append((r, n))
        r += n
    return out


# =================================================================== device

def build_program():
    nc = bacc.Bacc()

    enc_in = nc.dram_tensor("enc_in", [2, 48, 128 * 128], f32r, kind="ExternalInput")
    w1 = nc.dram_tensor("w1", [48, 64], f32r, kind="ExternalInput")
    w2 = nc.dram_tensor("w2", [4 * 2 * P, 128], f32r, kind="ExternalInput")
    w3 = nc.dram_tensor("w3", [4 * 4 * P, 256], f32r, kind="ExternalInput")
    w4 = nc.dram_tensor("w4", [4 * 8 * P, 512], f32r, kind="ExternalInput")
    wx_l = [nc.dram_tensor(f"wx{l}", [9 * 4 * P, 256], f32r, kind="ExternalInput") for l in range(3)]
    wh_l = [nc.dram_tensor(f"wh{l}", [9 * 4 * P, 256], f32r, kind="ExternalInput") for l in range(3)]
    lb_l = [nc.dram_tensor(f"lb{l}", [2 * P, 1], f32, kind="ExternalInput") for l in range(3)]
    wg1 = nc.dram_tensor("wg1", [9 * 4 * P, 1024], f32r, kind="ExternalInput")
    bg1 = nc.dram_tensor("bg1", [8 * P, 1], f32, kind="ExternalInput")
    wg2 = [nc.dram_tensor(f"wg2_{g}", [9 * 4 * P, 256], f32r, kind="ExternalInput") for g in range(2)]
    bg2 = [nc.dram_tensor(f"bg2_{g}", [2 * P, 1], f32, kind="ExternalInput") for g in range(2)]
    wg3 = [nc.dram_tensor(f"wg3_{g}", [9 * 2 * P, 128], f32r, kind="ExternalInput") for g in range(2)]
    bg3 = [nc.dram_tensor(f"bg3_{g}", [P, 1], f32, kind="ExternalInput") for g in range(2)]
    wg4 = [nc.dram_tensor(f"wg4_{g}", [9 * P, 64], f32r, kind="ExternalInput") for g in range(2)]
    bg4 = [nc.dram_tensor(f"bg4_{g}", [64, 1], f32, kind="ExternalInput") for g in range(2)]
    wfin = nc.dram_tensor("wfin", [9 * P, 5], f32r, kind="ExternalInput")
    bfin = nc.dram_tensor("bfin", [5, 1], f32, kind="ExternalInput")
    wfuse = nc.dram_tensor("wfuse", [6, 3], f32r, kind="ExternalInput")
    bfuse = nc.dram_tensor("bfuse", [3, 1], f32, kind="ExternalInput")
    wpad = nc.dram_tensor("wpad", [3, 130, 258], f32, kind="ExternalInput")
    yflag = nc.dram_tensor("yflag", [1, 1], f32, kind="ExternalInput")

    frame_out = nc.dram_tensor("frame_out", [3, 128, 256], f32, kind="ExternalOutput")
    flow_out = nc.dram_tensor("flow_out", [2, 128, 256], f32, kind="ExternalOutput")
    pred_out = nc.dram_tensor("pred_out", [3, 128, 256], f32, kind="ExternalOutput")
    dbg_feat = (nc.dram_tensor("dbg_feat", [16, 512, 256], f32, kind="ExternalOutput")
                if STAGE == "enc" else None)
    dbg_h = (nc.dram_tensor("dbg_h", [512, B, 256], f32, kind="ExternalOutput")
             if STAGE == "lstm" else None)

    with tile.TileContext(nc) as tc:
        with (
            tc.tile_pool(name="persist_dram", bufs=1, space="DRAM") as pdram,
            tc.tile_pool(name="zp", bufs=1) as zpool,
        ):
            zero128 = zpool.tile([P, 1024], f32)
            nc.vector.memset(zero128[:], 0.0)
            zdram = pdram.tile([P, 1024], f32r, tag="zdram")
            nc.sync.dma_start(zdram[:], zero128[:].bitcast(f32r))

            def zero_fill(ap):
                flat = ap  # expects [128, N] contiguous view
                n = flat.shape[-1]
                for n0 in range(0, n, 1024):
                    w_ = min(1024, n - n0)
                    nc.sync.dma_start(flat[:, n0:n0 + w_], zdram[:, :w_])

            gath_feat = pdram.tile([16, 512, 256], f32r)
            gath_h = [[pdram.tile([512, B, 256], f32r, tag=f"gh{l}_{t}", name=f"gh{l}_{t}")
                       for t in range(T)] for l in range(3)]
            a2a_recv = pdram.tile([8, 64, 12, 16], f32r)

            build_encoder(nc, tc, enc_in, w1, w2, w3, w4, gath_feat, pdram, zero128)
            if STAGE == "enc":
                nc.sync.dma_start(dbg_feat.ap(), gath_feat[:].bitcast(f32))
            if STAGE != "enc":
                build_lstm(nc, tc, wx_l, wh_l, lb_l, gath_feat, gath_h, a2a_recv,
                           pdram, dbg_h, zero_fill)
            if STAGE in ("gen", "full"):
                build_generators(nc, tc, a2a_recv, wg1, bg1, wg2, bg2, wg3, bg3,
                                 wg4, bg4, wfin, bfin, wfuse, bfuse, wpad, yflag,
                                 frame_out, flow_out, pred_out, zero128, zero_fill)
            else:
                for t_ in (frame_out, flow_out, pred_out):
                    tv = t_.ap().rearrange("c y x -> c (y x)")
                    for n0 in range(0, 128 * 256, 1024):
                        nc.sync.dma_start(tv[:, n0:n0 + 1024], zero128[:t_.shape[0], :])
    nc.compile()
    return nc


def build_encoder(nc, tc, enc_in, w1, w2, w3, w4, gath_feat, pdram, zero128):
    with (
        tc.tile_pool(name="enc_w", bufs=1) as wp,
        tc.tile_pool(name="enc_w4", bufs=2) as wp4,
        tc.tile_pool(name="enc_sb", bufs=3) as sb,
        tc.tile_pool(name="enc_pl", bufs=1) as plp,
        tc.tile_pool(name="enc_pl4", bufs=2) as plp4,
        tc.tile_pool(name="enc_ps", bufs=4, space="PSUM") as ps,
        tc.tile_pool(name="enc_ps4", bufs=4, space="PSUM") as ps4,
        tc.tile_pool(name="enc_dram", bufs=1, space="DRAM") as edram,
    ):
        alpha = wp.tile([P, 1], f32, tag="alpha")
        nc.vector.memset(alpha[:], 0.2)
        e1 = [edram.tile([64, 130, 130], f32r, tag=f"e1_{j}", name=f"e1_{j}") for j in range(2)]
        e2 = [edram.tile([128, 66, 66], f32r, tag=f"e2_{j}", name=f"e2_{j}") for j in range(2)]
        e3 = [edram.tile([256, 34, 34], f32r, tag=f"e3_{j}", name=f"e3_{j}") for j in range(2)]
        enc_out = pdram.tile([2, 512, 256], f32r)
        for j in range(2):
            for buf, cch, hp_ in ((e1[j], 64, 130), (e2[j], 128, 66), (e3[j], 256, 34)):
                for cc0 in range(0, cch, P):
                    cn = min(P, cch - cc0)
                    z = zero128[:cn, :1]
                    nc.sync.dma_start(buf[cc0:cc0 + cn, 0, :].bitcast(f32), zero128[:cn, :hp_])
                    nc.sync.dma_start(buf[cc0:cc0 + cn, hp_ - 1, :].bitcast(f32), zero128[:cn, :hp_])
                    nc.sync.dma_start(buf[cc0:cc0 + cn, 1:hp_ - 1, 0].bitcast(f32), zero128[:cn, :hp_ - 2])
                    nc.sync.dma_start(buf[cc0:cc0 + cn, 1:hp_ - 1, hp_ - 1].bitcast(f32), zero128[:cn, :hp_ - 2])

        # conv1 (K=48 host-im2col)
        w1t = wp.tile([48, 64], f32r, tag="w1")
        nc.sync.dma_start(w1t[:], w1.ap())
        for j in range(2):
            imt = plp.tile([48, 128 * 128], f32r, tag="im2col")
            nc.sync.dma_start(imt[:], enc_in.ap()[j])
            for r0, nr in row_tiles(128, 128, 512):
                pt = ps.tile([64, 512], f32, tag="eps")
                nc.tensor.matmul(pt[:, :nr * 128], w1t[:], imt[:, r0 * 128:(r0 + nr) * 128],
                                 start=True, stop=True)
                ot = sb.tile([64, 512], f32, tag="c1o")
                nc.scalar.activation(ot[:, :nr * 128], pt[:, :nr * 128], AF.Prelu, alpha=alpha[:64])
                nc.sync.dma_start(e1[j][:, 1 + r0:1 + r0 + nr, 1:129],
                                  ot[:, :nr * 128].rearrange("c (y x) -> c y x", y=nr).bitcast(f32r))

        # conv2 / conv3 (shifts inner, weights fully resident)
        for li, (wdram, src, dst, cin, hin2, hout, mout) in enumerate((
                (w2, e1, e2, 64, 65, 64, 128),
                (w3, e2, e3, 128, 33, 32, 256))):
            kch = 4 * cin // P
            wt = wp.tile([P, 4 * kch, mout], f32r, tag=f"wenc{li}")
            nc.sync.dma_start(wt[:], wdram.ap().rearrange("(sk p) m -> p sk m", p=P))
            for j in range(2):
                xt = plp.tile([P, kch, hin2, hin2], f32r, tag=f"pl{li}")
                srcr = src[j][:].rearrange("c (i py) (j2 px) -> c py px i j2", py=2, px=2)
                for pl in range(4):
                    ry, rx = pl // 2, pl % 2
                    if cin == 64:
                        nc.sync.dma_start(xt[(pl % 2) * 64:(pl % 2) * 64 + 64, pl // 2],
                                          srcr[:, ry, rx])
                    else:
                        nc.sync.dma_start(xt[:, pl], srcr[:, ry, rx])
                for m in range(mout // P):
                    for r0, nr in row_tiles(hout, hout, 512):
                        npx = nr * hout
                        pt = ps.tile([P, 512], f32, tag="eps")
                        first = True
                        for s in range(4):
                            qy, qx = s // 2, s % 2
                            for k in range(kch):
                                nc.tensor.matmul(
                                    pt[:, :npx], wt[:, s * kch + k, m * P:(m + 1) * P],
                                    xt[:, k, qy + r0:qy + r0 + nr, qx:qx + hout],
                                    start=first, stop=(s == 3 and k == kch - 1))
                                first = False
                        ot = sb.tile([P, 512], f32, tag=f"c{li}o")
                        nc.scalar.activation(ot[:, :npx], pt[:, :npx], AF.Prelu, alpha=alpha[:])
                        nc.sync.dma_start(
                            dst[j][m * P:(m + 1) * P, 1 + r0:1 + r0 + nr, 1:1 + hout],
                            ot[:, :npx].rearrange("c (y x) -> c y x", y=nr).bitcast(f32r))

        # conv4: j outer, shifts outer (PSUM held across shifts), w4 loaded per shift
        for j in range(2):
            xt = plp4.tile([P, 8, 17, 17], f32r, tag="pl3")
            srcr = e3[j][:].rearrange("c (i py) (j2 px) -> c py px i j2", py=2, px=2)
            for pl in range(4):
                ry, rx = pl // 2, pl % 2
                for hh in range(2):
                    nc.sync.dma_start(xt[:, pl * 2 + hh], srcr[hh * P:(hh + 1) * P, ry, rx])
            pts = [ps4.tile([P, 256], f32, tag="e4ps", name=f"e4ps_{j}_{_i}") for _i in range(4)]
            for s in range(4):
                qy, qx = s // 2, s % 2
                wt4 = wp4.tile([P, 8, 512], f32r, tag="w4s")
                nc.sync.dma_start(
                    wt4[:], w4.ap().rearrange("(sk p) m -> p sk m", p=P)[:, s * 8:(s + 1) * 8, :])
                for m in range(4):
                    for k in range(8):
                        nc.tensor.matmul(
                            pts[m][:], wt4[:, k, m * P:(m + 1) * P],
                            xt[:, k, qy:qy + 16, qx:qx + 16],
                            start=(s == 0 and k == 0), stop=(s == 3 and k == 7))
            for m in range(4):
                ot = sb.tile([P, 256], f32, tag="c4o")
                nc.scalar.activation(ot[:], pts[m][:], AF.Prelu, alpha=alpha[:])
                nc.sync.dma_start(enc_out[j, m * P:(m + 1) * P, :], ot[:].bitcast(f32r))

        nc.gpsimd.collective_compute(
            "AllGather", OP.bypass, replica_groups=[list(range(NCORES))],
            ins=[enc_out[:].opt()], outs=[gath_feat[:].opt()])


def build_lstm(nc, tc, wx_l, wh_l, lb_l, gath_feat, gath_h, a2a_recv, pdram, dbg_h, zero_fill):
    NPX = B * 256
    with (
        tc.tile_pool(name="lstm_w", bufs=1) as wp,
        tc.tile_pool(name="lstm_sb", bufs=1) as sb,
        tc.tile_pool(name="lstm_gx", bufs=1) as gxp,
        tc.tile_pool(name="lstm_pl", bufs=1) as plp,
        tc.tile_pool(name="lstm_ps", bufs=4, space="PSUM") as ps,
        tc.tile_pool(name="lstm_dram", bufs=2, space="DRAM") as ldram,
    ):
        h_last = None
        for l in range(KN_LAYERS):
            wx = wp.tile([P, 36, 256], f32r, tag="wx")
            wh = wp.tile([P, 36, 256], f32r, tag="wh")
            nc.sync.dma_start(wx[:], wx_l[l].ap().rearrange("(tk p) m -> p tk m", p=P))
            nc.sync.dma_start(wh[:], wh_l[l].ap().rearrange("(tk p) m -> p tk m", p=P))
            bias = wp.tile([P, 2, 1], f32, tag="lbias")
            nc.sync.dma_start(bias[:], lb_l[l].ap().rearrange("(ch p) o -> p ch o", p=P))

            xp = plp.tile([P, 4, B, 18, 18], f32r, tag="xp")
            hp = plp.tile([P, 4, B, 18, 18], f32r, tag="hp")
            zero_fill(xp[:].rearrange("p a b c d -> p (a b c d)"))
            zero_fill(hp[:].rearrange("p a b c d -> p (a b c d)"))
            gx = gxp.tile([P, 2, KN_STEPS, NPX], f32, tag="gx")

            for t in range(KN_STEPS):
                for k in range(4):
                    for b in range(B):
                        if l == 0:
                            src = gath_feat[b * 4 + t, k * P:(k + 1) * P, :]
                        else:
                            src = gath_h[l - 1][t][k * P:(k + 1) * P, b, :]
                        nc.sync.dma_start(xp[:, k, b, 1:17, 1:17],
                                          src.rearrange("c (y x) -> c y x", y=16))
                for m in range(2):
                    for nh in range(2):
                        pt = ps.tile([P, 512], f32, tag="lps")
                        first = True
                        for tap in range(9):
                            ky, kx = tap // 3, tap % 3
                            for k in range(4):
                                nc.tensor.matmul(
                                    pt[:], wx[:, tap * 4 + k, m * P:(m + 1) * P],
                                    xp[:, k, nh * 2:nh * 2 + 2, ky:ky + 16, kx:kx + 16],
                                    start=first, stop=(tap == 8 and k == 3))
                                first = False
                        nc.scalar.activation(gx[:, m, t, nh * 512:(nh + 1) * 512], pt[:],
                                             AF.Identity, bias=bias[:, m])

            c_t = sb.tile([P, NPX], f32, tag="c_t")
            for t in range(KN_STEPS):
                if t > 0:
                    for k in range(4):
                        for b in range(B):
                            src = gath_h[l][t - 1][k * P:(k + 1) * P, b, :]
                            nc.sync.dma_start(hp[:, k, b, 1:17, 1:17],
                                              src.rearrange("c (y x) -> c y x", y=16))
                    sAB = []
                    for m in range(2):
                        sm = sb.tile([P, NPX], f32, tag=f"s{m}")
                        for nh in range(2):
                            pt = ps.tile([P, 512], f32, tag="lps")
                            first = True
                            for tap in range(9):
                                ky, kx = tap // 3, tap % 3
                                for k in range(4):
                                    nc.tensor.matmul(
                                        pt[:], wh[:, tap * 4 + k, m * P:(m + 1) * P],
                                        hp[:, k, nh * 2:nh * 2 + 2, ky:ky + 16, kx:kx + 16],
                                        start=first, stop=(tap == 8 and k == 3))
                                    first = False
                            nc.vector.tensor_add(sm[:, nh * 512:(nh + 1) * 512], pt[:],
                                                 gx[:, m, t, nh * 512:(nh + 1) * 512])
                        sAB.append(sm)
                    sA, sB_ = sAB
                else:
                    sA = gx[:, 0, 0]
                    sB_ = gx[:, 1, 0]
                nc.scalar.activation(sA[:], sA[:], AF.Sigmoid)        # [sig(i)|sig(f)]
                nc.scalar.activation(sB_[0:64], sB_[0:64], AF.Tanh)    # tanh(g)
                nc.scalar.activation(sB_[64:128], sB_[64:128], AF.Sigmoid)  # sig(o)
                it = sb.tile([64, NPX], f32, tag="it")
                nc.vector.tensor_mul(it[:], sA[0:64], sB_[0:64])
                it_hi = sb.tile([P, NPX], f32, tag="ithi")
                nc.sync.dma_start(it_hi[64:128], it[:])
                if t > 0:
                    nc.vector.tensor_mul(c_t[64:128], sA[64:128], c_t[64:128])
                    nc.vector.tensor_add(c_t[64:128], c_t[64:128], it_hi[64:128])
                else:
                    nc.vector.tensor_copy(c_t[64:128], it_hi[64:128])
                tct = sb.tile([P, NPX], f32, tag="tct")
                nc.scalar.activation(tct[64:128], c_t[64:128], AF.Tanh)
                h_t = sb.tile([P, NPX], f32, tag="h_t")
                nc.vector.tensor_mul(h_t[64:128], sB_[64:128], tct[64:128])
                cc_in = ldram.tile([64, B, 256], f32r, tag="ccin")
                nc.sync.dma_start(cc_in[:],
                                  h_t[64:128].rearrange("c (b px) -> c b px", b=B).bitcast(f32r))
                nc.gpsimd.collective_compute(
                    "AllGather", OP.bypass, replica_groups=[list(range(NCORES))],
                    ins=[cc_in[:].opt()], outs=[gath_h[l][t][:].opt()])
                if l == KN_LAYERS - 1 and t == KN_STEPS - 1:
                    h_last = h_t
        if dbg_h is not None:
            nc.sync.dma_start(dbg_h.ap(), gath_h[KN_LAYERS - 1][KN_STEPS - 1][:].bitcast(f32))

        # AllToAll out_me slab distribution
        send = sb.tile([64, 8, 12 * 16], f32, tag="send")
        nc.vector.memset(send[:], 0.0)
        hl = h_last[64:128].rearrange("c (b y x) -> c b y x", b=B, y=16)
        for d in range(8):
            s_d, cp = d // 2, d % 2
            z0, z1 = (2, 12) if cp == 0 else (0, 10)
            img0 = 8 * cp - 2 + z0
            nc.sync.dma_start(
                send[:, d, z0 * 16:z1 * 16], hl[:, s_d, img0:img0 + (z1 - z0), :])
        a2a_send = ldram.tile([8, 64, 12, 16], f32r, tag="a2asend")
        nc.sync.dma_start(a2a_send[:].rearrange("d c z x -> c d (z x)"),
                          send[:].bitcast(f32r))
        nc.gpsimd.collective_compute(
            "AllToAll", OP.bypass, replica_groups=[list(range(NCORES))],
            ins=[a2a_send[:].opt()], outs=[a2a_recv[:].opt()])


def build_generators(nc, tc, a2a_recv, wg1, bg1, wg2, bg2, wg3, bg3, wg4, bg4,
                     wfin, bfin, wfuse, bfuse, wpad, yflag,
                     frame_out, flow_out, pred_out, zero128, zero_fill):
    with (
        tc.tile_pool(name="gen_w", bufs=1) as wp,
        tc.tile_pool(name="gen_w1", bufs=2) as wp1,
        tc.tile_pool(name="gen_sb", bufs=1) as sb,
        tc.tile_pool(name="gen_pl", bufs=1) as plp,
        tc.tile_pool(name="gen_ps", bufs=4, space="PSUM") as ps,
        tc.tile_pool(name="gen_dram", bufs=1, space="DRAM") as gdram,
    ):
        # TRUE-interleaved images in DRAM (+1 col pad to avoid AP merging).
        def ibuf(nm, cch, rr, cc):
            return [gdram.tile([cch, rr, cc + 1], f32r, tag=f"{nm}_{g}", name=f"{nm}_{g}")
                    for g in range(2)]
        s1 = ibuf("s1", 512, 32, 32)
        s2 = ibuf("s2", 256, 36, 64)
        s3 = ibuf("s3", 128, 68, 128)
        s4 = ibuf("s4", 64, 132, 256)
        warp_dram = gdram.tile([3, 128, 256], f32r, tag="warp_dram")

        def store_phase(dstbuf, cs0, csz, py, px, rt0, rn, win, stg):
            # per-row DMAs: dst [c, win step-2 cols], src [c, win]
            for r in range(rn):
                nc.sync.dma_start(
                    dstbuf[cs0:cs0 + csz, 2 * (rt0 + r) + py, px:2 * win - 1 + px:2],
                    stg[:csz, r, :win].bitcast(f32r))

        # ---- L1 merged (M = 1024) ----
        zt = plp.tile([P, 4, 17, 17], f32r, tag="zt")
        zero_fill(zt[:].rearrange("p a b c -> p (a b c)"))
        for k in range(4):
            for half in range(2):
                nc.sync.dma_start(zt[half * 64:half * 64 + 64, k, 0:12, 0:16],
                                  a2a_recv[k * 2 + half])
        b1b = wp.tile([P, 8, 1], f32, tag="bias1")
        nc.sync.dma_start(b1b[:], bg1.ap().rearrange("(mm p) o -> p mm o", p=P))
        for py in (0, 1):
            for px in (0, 1):
                taps = [(ky, kx, dy, dx) for (a_, b_, ky, kx, dy, dx) in DEC_TAPS
                        if a_ == py and b_ == px]
                for mg in range(2):
                    pts = [ps.tile([P, 16, 16], f32, tag="gps", name=f"l1p_{py}{px}{mg}{_i}")
                           for _i in range(4)]
                    for ti, (ky, kx, dy, dx) in enumerate(taps):
                        tap_idx = DEC_TAPS.index((py, px, ky, kx, dy, dx))
                        wt = wp1.tile([P, 4, 512], f32r, tag="wg1t")
                        nc.sync.dma_start(
                            wt[:],
                            wg1.ap().rearrange("(tk p) m -> p tk m", p=P)[
                                :, tap_idx * 4:(tap_idx + 1) * 4, mg * 512:(mg + 1) * 512])
                        for k in range(4):
                            for mi in range(4):
                                nc.tensor.matmul(
                                    pts[mi][:].rearrange("c y x -> c (y x)"),
                                    wt[:, k, mi * P:(mi + 1) * P],
                                    zt[:, k, dy:dy + 16, dx:dx + 16],
                                    start=(ti == 0 and k == 0),
                                    stop=(ti == len(taps) - 1 and k == 3))
                    for mi in range(4):
                        m = mg * 4 + mi
                        g, mm_ = m // 4, m % 4
                        stg = sb.tile([P, 16, 17], f32, tag="l1stg")
                        nc.scalar.activation(stg[:, :, :16], pts[mi][:], AF.Relu,
                                             bias=b1b[:, m])
                        store_phase(s1[g], mm_ * P, P, py, px, 0, 16, 16, stg)

        # ---- L2..L4 per gen ----
        for (wd, bd, srcs, r0_, r1_, dsts, cin, win, mout) in (
                (wg2, bg2, s1, 3, 21, s2, 512, 32, 256),
                (wg3, bg3, s2, 1, 35, s3, 256, 64, 128),
                (wg4, bg4, s3, 1, 67, s4, 128, 128, 64)):
            kch = cin // P
            nrows = r1_ - r0_
            msize = min(P, mout)
            mchunks = mout // msize
            rn_max = max(1, 512 // win)
            for g in range(2):
                wt = wp.tile([P, 9 * kch, mout], f32r, tag=f"wg_{cin}")
                nc.sync.dma_start(wt[:], wd[g].ap().rearrange("(tk p) m -> p tk m", p=P))
                bt = wp.tile([msize, mchunks, 1], f32, tag=f"bg_{cin}")
                nc.sync.dma_start(bt[:], bd[g].ap().rearrange("(mm p) o -> p mm o", p=msize))
                xt = plp.tile([P, kch, nrows + 1, win + 1], f32r, tag=f"gpl_{cin}")
                zero_fill(xt[:].rearrange("p a b c -> p (a b c)"))
                for k in range(kch):
                    nc.sync.dma_start(xt[:, k, 0:nrows, 0:win],
                                      srcs[g][k * P:(k + 1) * P, r0_:r1_, :win])
                for py in (0, 1):
                    for px in (0, 1):
                        taps = [(ky, kx, dy, dx) for (a_, b_, ky, kx, dy, dx) in DEC_TAPS
                                if a_ == py and b_ == px]
                        for m in range(mchunks):
                            for rt0, rn in row_tiles(nrows, win, 512):
                                npx = rn * win
                                pt = ps.tile([P, rn_max, win], f32, tag="gps")
                                first = True
                                for ti, (ky, kx, dy, dx) in enumerate(taps):
                                    tap_idx = DEC_TAPS.index((py, px, ky, kx, dy, dx))
                                    for k in range(kch):
                                        nc.tensor.matmul(
                                            pt[:msize, :rn].rearrange("c y x -> c (y x)"),
                                            wt[:, tap_idx * kch + k, m * msize:(m + 1) * msize],
                                            xt[:, k, dy + rt0:dy + rt0 + rn, dx:dx + win],
                                            start=first,
                                            stop=(ti == len(taps) - 1 and k == kch - 1))
                                        first = False
                                stg = sb.tile([P, rn_max, win + 1], f32, tag="gstg")
                                nc.scalar.activation(stg[:msize, :rn, :win],
                                                     pt[:msize, :rn], AF.Relu, bias=bt[:, m])
                                store_phase(dsts[g], m * msize, msize, py, px,
                                            rt0, rn, win, stg)

        # ---- final conv (frame||flow K-stacked, M=5) ----
        wfin_t = wp.tile([P, 9, 5], f32r, tag="wfin")
        nc.sync.dma_start(wfin_t[:], wfin.ap().rearrange("(t p) m -> p t m", p=P))
        bfin_t = wp.tile([5, 1], f32, tag="bfin")
        nc.sync.dma_start(bfin_t[:], bfin.ap())
        fin_pl = plp.tile([P, 12, 258], f32r, tag="fin_pl")
        zero_fill(fin_pl[:].rearrange("p a b -> p (a b)"))
        for rt0 in range(0, 128, 10):
            rn = min(10, 128 - rt0)
            for g in range(2):
                nc.sync.dma_start(fin_pl[g * 64:g * 64 + 64, 0:rn + 2, 1:257],
                                  s4[g][:, rt0:rt0 + rn + 2, :256])
            for st0, sn in row_tiles(rn, 256, 512):
                npx = sn * 256
                pt = ps.tile([P, 512], f32, tag="gps")
                first = True
                for tap in range(9):
                    ky, kx = tap // 3, tap % 3
                    nc.tensor.matmul(pt[:5, :npx], wfin_t[:, tap, :],
                                     fin_pl[:, st0 + ky:st0 + ky + sn, kx:kx + 256],
                                     start=first, stop=(tap == 8))
                    first = False
                sig = sb.tile([5, 512], f32, tag="finsig")
                nc.scalar.activation(sig[:, :npx], pt[:5, :npx], AF.Sigmoid, bias=bfin_t[:])
                tnh = sb.tile([5, 512], f32, tag="fintanh")
                nc.scalar.activation(tnh[:, :npx], pt[:5, :npx], AF.Tanh, bias=bfin_t[:])
                rr = rt0 + st0
                nc.sync.dma_start(frame_out.ap()[:, rr:rr + sn, :],
                                  sig[0:3, :npx].rearrange("c (y x) -> c y x", y=sn))
                nc.sync.dma_start(flow_out.ap()[:, rr:rr + sn, :],
                                  tnh[3:5, :npx].rearrange("c (y x) -> c y x", y=sn))

        build_warp(nc, sb, wp, wpad, yflag, flow_out, warp_dram)

        # fuse 1x1 conv + sigmoid
        wfu = wp.tile([6, 3], f32r, tag="wfu")
        nc.sync.dma_start(wfu[:], wfuse.ap())
        bfu = wp.tile([3, 1], f32, tag="bfu")
        nc.sync.dma_start(bfu[:], bfuse.ap())
        frame_flat = frame_out.ap().rearrange("c y x -> c (y x)")
        warp_flat = warp_dram[:].rearrange("c y x -> c (y x)")
        for n0 in range(0, 128 * 256, 512):
            ft = sb.tile([6, 512], f32r, tag="fusein")
            nc.sync.dma_start(ft[0:3, :], frame_flat[:, n0:n0 + 512].bitcast(f32r))
            nc.sync.dma_start(ft[3:6, :], warp_flat[:, n0:n0 + 512])
            pt = ps.tile([P, 512], f32, tag="gps")
            nc.tensor.matmul(pt[:3], wfu[:], ft[:], start=True, stop=True)
            ot = sb.tile([3, 512], f32, tag="fuseo")
            nc.scalar.activation(ot[:], pt[:3], AF.Sigmoid, bias=bfu[:])
            nc.sync.dma_start(pred_out.ap().rearrange("c y x -> c (y x)")[:, n0:n0 + 512], ot[:])


def build_warp(nc, sb, wp, wpad, yflag, flow_out, warp_dram):
    yfl = wp.tile([1, 1], f32, tag="yfl")
    nc.sync.dma_start(yfl[:], yflag.ap())
    for chunk in range(2):
        r0 = chunk * 64
        fx = sb.tile([64, 256], f32, tag="wfx")
        fy = sb.tile([64, 256], f32, tag="wfy")
        nc.sync.dma_start(fx[:], flow_out.ap()[0, r0:r0 + 64, :])
        nc.sync.dma_start(fy[:], flow_out.ap()[1, r0:r0 + 64, :])
        planes = {}
        for nm, f_ in (("x", fx), ("y", fy)):
            pp = sb.tile([64, 256], f32, tag=f"w{nm}p")
            mm_ = sb.tile([64, 256], f32, tag=f"w{nm}m")
            zz = sb.tile([64, 256], f32, tag=f"w{nm}0")
            nc.scalar.activation(pp[:], f_[:], AF.Relu)
            nc.scalar.activation(mm_[:], f_[:], AF.Relu, scale=-1.0)
            nc.scalar.activation(zz[:], f_[:], AF.Abs)
            nc.vector.tensor_scalar(zz[:], zz[:], -1.0, 1.0, OP.mult, OP.add)
            planes[nm] = (pp, mm_, zz)
        wxp, wxm, wx0 = planes["x"]
        wyp, wym, wy0 = planes["y"]
        # col-0 / row-0 correction factors
        tcol = sb.tile([64, 1], f32, tag="tcol")
        mneg = sb.tile([64, 1], f32, tag="mneg")
        nc.vector.tensor_scalar(mneg[:], fx[:, 0:1], 0.0, None, OP.is_lt)
        nc.vector.tensor_scalar(tcol[:], fx[:, 0:1], 1.0, None, OP.add)
        nc.vector.tensor_mul(tcol[:], tcol[:], mneg[:])
        trow = sb.tile([1, 256], f32, tag="trow")
        mrow = sb.tile([1, 256], f32, tag="mrow")
        nc.vector.tensor_scalar(mrow[:], fy[0:1, :], 0.0, None, OP.is_lt)
        nc.vector.tensor_scalar(trow[:], fy[0:1, :], 1.0, None, OP.add)
        nc.vector.tensor_mul(trow[:], trow[:], mrow[:])
        nc.vector.tensor_scalar_mul(trow[:], trow[:], yfl[:1])

        for ch in range(3):
            wrow = sb.tile([66, 258], f32, tag="wrow")
            nc.sync.dma_start(wrow[:], wpad.ap()[ch, r0:r0 + 66, :])
            shifted = []
            for si, nm in ((0, "im"), (1, "i0"), (2, "ip")):
                tl = sb.tile([64, 258], f32, tag=nm)
                nc.sync.dma_start(tl[:], wrow[si:si + 64, :])
                shifted.append(tl)
            X = []
            for img, nm in zip(shifted, ("xm", "x0", "xp")):
                xi = sb.tile([64, 256], f32, tag=f"X{nm}")
                tmp = sb.tile([64, 256], f32, tag=f"Xt{nm}")
                nc.vector.tensor_mul(xi[:], wx0[:], img[:, 1:257])
                nc.vector.tensor_mul(tmp[:], wxp[:], img[:, 2:258])
                nc.vector.tensor_add(xi[:], xi[:], tmp[:])
                nc.vector.tensor_mul(tmp[:], wxm[:], img[:, 0:256])
                nc.vector.tensor_add(xi[:], xi[:], tmp[:])
                d01 = sb.tile([64, 1], f32, tag=f"d{nm}")
                nc.vector.tensor_tensor(d01[:], img[:, 2:3], img[:, 1:2], OP.subtract)
                nc.vector.tensor_mul(d01[:], d01[:], tcol[:])
                nc.vector.tensor_add(xi[:, 0:1], xi[:, 0:1], d01[:])
                X.append(xi)
            Xm, X0, Xp = X
            v = sb.tile([64, 256], f32, tag="wv")
            tmp2 = sb.tile([64, 256], f32, tag="wvt")
            nc.vector.tensor_mul(v[:], wy0[:], X0[:])
            nc.vector.tensor_mul(tmp2[:], wyp[:], Xp[:])
            nc.vector.tensor_add(v[:], v[:], tmp2[:])
            nc.vector.tensor_mul(tmp2[:], wym[:], Xm[:])
            nc.vector.tensor_add(v[:], v[:], tmp2[:])
            if chunk == 0:
                dr = sb.tile([1, 256], f32, tag="dr")
                nc.vector.tensor_tensor(dr[:], Xp[0:1, :], X0[0:1, :], OP.subtract)
                nc.vector.tensor_mul(dr[:], dr[:], trow[:])
                nc.vector.tensor_add(v[0:1, :], v[0:1, :], dr[:])
            nc.sync.dma_start(warp_dram[ch, r0:r0 + 64, :], v[:].bitcast(f32r))


# ==================================================================== host

_CACHE = {}


def _prep_weights(cnn_ws, lstm_ws, lstm_bs, gframe_ws, gframe_bs, gflow_ws,
                  gflow_bs, fuse_w, fuse_b):
    f = np.float32
    out = {}
    w1 = np.asarray(cnn_ws[0], f)          # (64, 3, 4, 4)
    out["w1"] = np.ascontiguousarray(w1.transpose(2, 3, 1, 0).reshape(48, 64))
    for li, nm in ((1, "w2"), (2, "w3"), (3, "w4")):
        w = np.asarray(cnn_ws[li], f)      # (Co, Ci, 4, 4)
        Co, Ci = w.shape[:2]
        shifts = []
        for qy in range(2):
            for qx in range(2):
                blocks = []
                for ry in range(2):
                    for rx in range(2):
                        blocks.append(w[:, :, 2 * qy + ry, 2 * qx + rx].T)  # (Ci, Co)
                shifts.append(np.concatenate(blocks, 0))                    # (4Ci, Co)
        out[nm] = np.ascontiguousarray(np.stack(shifts, 0).reshape(4 * 4 * Ci // P * P, Co))

    # per-core LSTM slices are produced later (need core index)
    out["_lstm_w"] = [np.asarray(w, f) for w in lstm_ws]
    out["_lstm_b"] = [np.asarray(b, f) for b in lstm_bs]

    def gen_layer(w):                       # deconv (Co, Ci, 3, 3) -> [9*KCH*128, Co]
        w = np.asarray(w, f)
        Co, Ci = w.shape[:2]
        taps = []
        for (py, ky, dy) in DEC_YT:
            for (px, kx, dx) in DEC_YT:
                taps.append(w[:, :, ky, kx].T)   # (Ci, Co)
        return np.ascontiguousarray(np.stack(taps, 0).reshape(9 * Ci, Co))

    wf, wl = [np.asarray(w, f) for w in gframe_ws], [np.asarray(w, f) for w in gflow_ws]
    bf = [np.asarray(b, f) for b in gframe_bs]
    bl = [np.asarray(b, f) for b in gflow_bs]
    out["wg1"] = np.concatenate([
        gen_layer(wf[0]).reshape(9, 512, 512), gen_layer(wl[0]).reshape(9, 512, 512)],
        axis=2).reshape(9 * 512, 1024)
    out["bg1"] = np.concatenate([bf[0], bl[0]])[:, None]
    for i, nm in ((1, "wg2"), (2, "wg3"), (3, "wg4")):
        out[f"{nm}_0"] = gen_layer(wf[i])
        out[f"{nm}_1"] = gen_layer(wl[i])
        out[f"b{nm[1:]}_0"] = bf[i][:, None]
        out[f"b{nm[1:]}_1"] = bl[i][:, None]
    # final conv: K = 64frame || 64flow stacked, M = 5
    wff, wlf = np.asarray(gframe_ws[4], f), np.asarray(gflow_ws[4], f)  # (3,64,3,3),(2,64,3,3)
    taps = []
    for ky in range(3):
        for kx in range(3):
            blk = np.zeros((128, 5), f)
            blk[0:64, 0:3] = wff[:, :, ky, kx].T
            blk[64:128, 3:5] = wlf[:, :, ky, kx].T
            taps.append(blk)
    out["wfin"] = np.concatenate(taps, 0)
    out["bfin"] = np.concatenate([np.asarray(gframe_bs[4], f),
                                  np.asarray(gflow_bs[4], f)])[:, None]
    out["wfuse"] = np.ascontiguousarray(np.asarray(fuse_w, f)[:, :, 0, 0].T)
    out["bfuse"] = np.asarray(fuse_b, f)[:, None]
    return out


def _lstm_core_slices(wl, bl, c):
    rows = []
    for gate in range(4):
        rows.extend(range(gate * 512 + 64 * c, gate * 512 + 64 * c + 64))
    # chunk order: [i|f] then [g|o]
    rows = np.array(rows[:128] + rows[128:], np.int64)
    ws = wl[rows]                     # (256, 1024, 3, 3)
    wx = ws[:, :512].transpose(2, 3, 1, 0).reshape(9, 512, 256).reshape(9 * 512, 256)
    whp = ws[:, 512:].transpose(2, 3, 1, 0).reshape(9, 512, 256).reshape(9 * 512, 256)
    bb = bl[rows][:, None]
    return (np.ascontiguousarray(wx), np.ascontiguousarray(whp),
            np.ascontiguousarray(bb))


def _im2col_conv1(frames):
    # frames: (2, 3, 256, 256) -> (2, 48, 128*128), tap-major rows (t*3+c)
    fpad = np.pad(frames, ((0, 0), (0, 0), (1, 1), (1, 1)))
    taps = []
    for dy in range(4):
        for dx in range(4):
            taps.append(fpad[:, :, dy:dy + 256:2, dx:dx + 256:2])
    arr = np.stack(taps, 1)  # (2, 16, 3, 128, 128)
    return np.ascontiguousarray(arr.reshape(2, 48, 128 * 128))


def kernel(x, cnn_ws, lstm_ws, lstm_bs, gframe_ws, gframe_bs, gflow_ws,
           gflow_bs, fuse_w, fuse_b):
    x = np.asarray(x, np.float32)
    wd = _prep_weights(cnn_ws, lstm_ws, lstm_bs, gframe_ws, gframe_bs,
                       gflow_ws, gflow_bs, fuse_w, fuse_b)

    frames = x.transpose(0, 2, 1, 3, 4).reshape(B * T, C, H, W)
    in_maps = []
    shared = {k: v for k, v in wd.items() if not k.startswith("_")}
    for cidx in range(NCORES):
        m = dict(shared)
        m["enc_in"] = _im2col_conv1(frames[2 * cidx:2 * cidx + 2])
        for l in range(3):
            wx, wh_, bb = _lstm_core_slices(wd["_lstm_w"][l], wd["_lstm_b"][l], cidx)
            m[f"wx{l}"], m[f"wh{l}"], m[f"lb{l}"] = wx, wh_, bb
        s, cp = cidx // 2, cidx % 2
        prev = x[s, :, -1]                         # (3, 256, 256)
        prow = np.pad(prev, ((0, 0), (1, 1), (1, 1)), mode="edge")  # (3,258,258)
        m["wpad"] = np.ascontiguousarray(prow[:, 128 * cp:128 * cp + 130, :])
        m["yflag"] = np.array([[1.0 if cp == 0 else 0.0]], np.float32)
        in_maps.append(m)

    if "nc" not in _CACHE:
        _CACHE["nc"] = build_program()
    nc = _CACHE["nc"]

    from concourse.bass_utils import run_bass_kernel_spmd
    res = run_bass_kernel_spmd(nc, in_maps, core_ids=list(range(NCORES)))
    results = res.results

    frame_pred = np.zeros((B, 3, H, W), np.float32)
    flow_pred = np.zeros((B, 2, H, W), np.float32)
    prediction = np.zeros((B, 3, H, W), np.float32)
    for cidx in range(NCORES):
        s, cp = cidx // 2, cidx % 2
        sl = slice(128 * cp, 128 * cp + 128)
        frame_pred[s, :, sl] = results[cidx]["frame_out"]
        flow_pred[s, :, sl] = results[cidx]["flow_out"]
        prediction[s, :, sl] = results[cidx]["pred_out"]
    if STAGE == "enc":
        return results[0]["dbg_feat"]
    if STAGE == "lstm":
        return results[0]["dbg_h"]
    return frame_pred, flow_pred, prediction


# revision 33
# speedup vs baseline: 1.0277x; 1.0277x over previous
"""DualMotionGAN forward on 8 Trainium2 NeuronCores (Bass/Tile, float32r matmuls).

Distribution: encoder data-parallel over the 16 frames (2 per core, stride-2
convs phase-decomposed into full-K tap matmuls); 3-layer ConvLSTM 8-way
gate-channel split (each core computes 64 channels of each gate for all
samples; h is AllGathered every step); an AllToAll then hands each core the
(sample, row-half) slab of out_me it needs, so the two deconv generators,
the bilinear warp (elementwise: flow=tanh in (-1,1) touches only the 3x3
neighborhood), and the fuse conv run spatially split with no further
communication. All per-core variation enters via host-sliced inputs --
the device program is identical on every core.
"""
import os
import numpy as np

import concourse.bass as bass
from concourse import bacc
import concourse.mybir as mybir
import concourse.tile as tile

f32r = mybir.dt.float32r
f32 = mybir.dt.float32
AF = mybir.ActivationFunctionType
OP = mybir.AluOpType
P = 128
NCORES = 8
B, C, T, H, W = 4, 3, 4, 256, 256

# deconv tap table: (phase, kernel index, input shift)
DEC_YT = [(0, 1, 0), (1, 0, 0), (1, 2, 1)]
DEC_TAPS = [(py, px, ky, kx, dy, dx)
            for (py, ky, dy) in DEC_YT for (px, kx, dx) in DEC_YT]

STAGE = os.environ.get("KSTAGE", "full")
KN_LAYERS = int(os.environ.get("KN_LAYERS", "3"))
KN_STEPS = int(os.environ.get("KN_STEPS", "4"))


def row_tiles(nrows, width, maxn=512):
    rpt = max(1, maxn // width)
    out = []
    r = 0
    while r < nrows:
        n = min(rpt, nrows - r)
        out.append((r, n))
        r += n
    return out


# =================================================================== device

def build_program():
    nc = bacc.Bacc()

    enc_in = nc.dram_tensor("enc_in", [2, 48, 128 * 128], f32r, kind="ExternalInput")
    w1 = nc.dram_tensor("w1", [48, 64], f32r, kind="ExternalInput")
    w2 = nc.dram_tensor("w2", [4 * 2 * P, 128], f32r, kind="ExternalInput")
    w3 = nc.dram_tensor("w3", [4 * 4 * P, 256], f32r, kind="ExternalInput")
    w4 = nc.dram_tensor("w4", [4 * 8 * P, 512], f32r, kind="ExternalInput")
    wx_l = [nc.dram_tensor(f"wx{l}", [9 * 4 * P, 256], f32r, kind="ExternalInput") for l in range(3)]
    wh_l = [nc.dram_tensor(f"wh{l}", [9 * 4 * P, 256], f32r, kind="ExternalInput") for l in range(3)]
    lb_l = [nc.dram_tensor(f"lb{l}", [2 * P, 1], f32, kind="ExternalInput") for l in range(3)]
    wg1 = nc.dram_tensor("wg1", [9 * 4 * P, 1024], f32r, kind="ExternalInput")
    bg1 = nc.dram_tensor("bg1", [8 * P, 1], f32, kind="ExternalInput")
    wg2 = [nc.dram_tensor(f"wg2_{g}", [9 * 4 * P, 256], f32r, kind="ExternalInput") for g in range(2)]
    bg2 = [nc.dram_tensor(f"bg2_{g}", [2 * P, 1], f32, kind="ExternalInput") for g in range(2)]
    wg3 = [nc.dram_tensor(f"wg3_{g}", [9 * 2 * P, 128], f32r, kind="ExternalInput") for g in range(2)]
    bg3 = [nc.dram_tensor(f"bg3_{g}", [P, 1], f32, kind="ExternalInput") for g in range(2)]
    wg4 = [nc.dram_tensor(f"wg4_{g}", [9 * P, 64], f32r, kind="ExternalInput") for g in range(2)]
    bg4 = [nc.dram_tensor(f"bg4_{g}", [64, 1], f32, kind="ExternalInput") for g in range(2)]
    wfin = nc.dram_tensor("wfin", [9 * P, 5], f32r, kind="ExternalInput")
    bfin = nc.dram_tensor("bfin", [5, 1], f32, kind="ExternalInput")
    wfuse = nc.dram_tensor("wfuse", [6, 3], f32r, kind="ExternalInput")
    bfuse = nc.dram_tensor("bfuse", [3, 1], f32, kind="ExternalInput")
    wpad = nc.dram_tensor("wpad", [3, 130, 258], f32, kind="ExternalInput")
    topmask = nc.dram_tensor("topmask", [64, 256], f32r, kind="ExternalInput")
    yflag = nc.dram_tensor("yflag", [1, 1], f32, kind="ExternalInput")

    frame_out = nc.dram_tensor("frame_out", [3, 128, 256], f32, kind="ExternalOutput")
    flow_out = nc.dram_tensor("flow_out", [2, 128, 256], f32, kind="ExternalOutput")
    pred_out = nc.dram_tensor("pred_out", [3, 128, 256], f32, kind="ExternalOutput")
    dbg_feat = (nc.dram_tensor("dbg_feat", [16, 512, 256], f32, kind="ExternalOutput")
                if STAGE == "enc" else None)
    dbg_h = (nc.dram_tensor("dbg_h", [512, B, 256], f32, kind="ExternalOutput")
             if STAGE == "lstm" else None)

    with tile.TileContext(nc) as tc:
        with (
            tc.tile_pool(name="persist_dram", bufs=1, space="DRAM") as pdram,
            tc.tile_pool(name="zp", bufs=1) as zpool,
        ):
            zero128 = zpool.tile([P, 1024], f32)
            nc.vector.memset(zero128[:], 0.0)
            zdram = pdram.tile([P, 1024], f32r, tag="zdram")
            nc.sync.dma_start(zdram[:], zero128[:].bitcast(f32r))

            def zero_fill(ap):
                flat = ap  # expects [128, N] contiguous view
                n = flat.shape[-1]
                for n0 in range(0, n, 1024):
                    w_ = min(1024, n - n0)
                    nc.sync.dma_start(flat[:, n0:n0 + w_], zdram[:, :w_])

            gath_feat = pdram.tile([16, 512, 256], f32r)
            gath_h = [[pdram.tile([512, B, 256], f32r, tag=f"gh{l}_{t}", name=f"gh{l}_{t}")
                       for t in range(T)] for l in range(3)]
            a2a_recv = pdram.tile([8, 64, 12, 16], f32r)

            build_encoder(nc, tc, enc_in, w1, w2, w3, w4, gath_feat, pdram, zero128)
            if STAGE == "enc":
                nc.sync.dma_start(dbg_feat.ap(), gath_feat[:].bitcast(f32))
            if STAGE != "enc":
                build_lstm(nc, tc, wx_l, wh_l, lb_l, gath_feat, gath_h, a2a_recv,
                           pdram, dbg_h, zero_fill)
            if STAGE in ("gen", "full"):
                build_generators(nc, tc, a2a_recv, wg1, bg1, wg2, bg2, wg3, bg3,
                                 wg4, bg4, wfin, bfin, wfuse, bfuse, wpad, yflag,
                                 topmask, frame_out, flow_out, pred_out, zero128, zero_fill)
            else:
                for t_ in (frame_out, flow_out, pred_out):
                    tv = t_.ap().rearrange("c y x -> c (y x)")
                    for n0 in range(0, 128 * 256, 1024):
                        nc.sync.dma_start(tv[:, n0:n0 + 1024], zero128[:t_.shape[0], :])
    nc.compile()
    return nc


def build_encoder(nc, tc, enc_in, w1, w2, w3, w4, gath_feat, pdram, zero128):
    with (
        tc.tile_pool(name="enc_w", bufs=1) as wp,
        tc.tile_pool(name="enc_w4", bufs=2) as wp4,
        tc.tile_pool(name="enc_sb", bufs=3) as sb,
        tc.tile_pool(name="enc_pl", bufs=1) as plp,
        tc.tile_pool(name="enc_pl4", bufs=2) as plp4,
        tc.tile_pool(name="enc_ps", bufs=4, space="PSUM") as ps,
        tc.tile_pool(name="enc_ps4", bufs=4, space="PSUM") as ps4,
        tc.tile_pool(name="enc_dram", bufs=1, space="DRAM") as edram,
    ):
        alpha = wp.tile([P, 1], f32, tag="alpha")
        nc.vector.memset(alpha[:], 0.2)
        e1 = [edram.tile([64, 130, 130], f32r, tag=f"e1_{j}", name=f"e1_{j}") for j in range(2)]
        e2 = [edram.tile([128, 66, 66], f32r, tag=f"e2_{j}", name=f"e2_{j}") for j in range(2)]
        e3 = [edram.tile([256, 34, 34], f32r, tag=f"e3_{j}", name=f"e3_{j}") for j in range(2)]
        enc_out = pdram.tile([2, 512, 256], f32r)
        for j in range(2):
            for buf, cch, hp_ in ((e1[j], 64, 130), (e2[j], 128, 66), (e3[j], 256, 34)):
                for cc0 in range(0, cch, P):
                    cn = min(P, cch - cc0)
                    z = zero128[:cn, :1]
                    nc.sync.dma_start(buf[cc0:cc0 + cn, 0, :].bitcast(f32), zero128[:cn, :hp_])
                    nc.sync.dma_start(buf[cc0:cc0 + cn, hp_ - 1, :].bitcast(f32), zero128[:cn, :hp_])
                    nc.sync.dma_start(buf[cc0:cc0 + cn, 1:hp_ - 1, 0].bitcast(f32), zero128[:cn, :hp_ - 2])
                    nc.sync.dma_start(buf[cc0:cc0 + cn, 1:hp_ - 1, hp_ - 1].bitcast(f32), zero128[:cn, :hp_ - 2])

        # conv1 (K=48 host-im2col)
        w1t = wp.tile([48, 64], f32r, tag="w1")
        nc.sync.dma_start(w1t[:], w1.ap())
        for j in range(2):
            imt = plp.tile([48, 128 * 128], f32r, tag="im2col")
            nc.sync.dma_start(imt[:], enc_in.ap()[j])
            for r0, nr in row_tiles(128, 128, 512):
                pt = ps.tile([64, 512], f32, tag="eps")
                nc.tensor.matmul(pt[:, :nr * 128], w1t[:], imt[:, r0 * 128:(r0 + nr) * 128],
                                 start=True, stop=True)
                ot = sb.tile([64, 512], f32, tag="c1o")
                nc.scalar.activation(ot[:, :nr * 128], pt[:, :nr * 128], AF.Prelu, alpha=alpha[:64])
                nc.sync.dma_start(e1[j][:, 1 + r0:1 + r0 + nr, 1:129],
                                  ot[:, :nr * 128].rearrange("c (y x) -> c y x", y=nr).bitcast(f32r))

        # conv2 / conv3 (shifts inner, weights fully resident)
        for li, (wdram, src, dst, cin, hin2, hout, mout) in enumerate((
                (w2, e1, e2, 64, 65, 64, 128),
                (w3, e2, e3, 128, 33, 32, 256))):
            kch = 4 * cin // P
            wt = wp.tile([P, 4 * kch, mout], f32r, tag=f"wenc{li}")
            nc.sync.dma_start(wt[:], wdram.ap().rearrange("(sk p) m -> p sk m", p=P))
            for j in range(2):
                xt = plp.tile([P, kch, hin2, hin2], f32r, tag=f"pl{li}")
                srcr = src[j][:].rearrange("c (i py) (j2 px) -> c py px i j2", py=2, px=2)
                for pl in range(4):
                    ry, rx = pl // 2, pl % 2
                    if cin == 64:
                        nc.sync.dma_start(xt[(pl % 2) * 64:(pl % 2) * 64 + 64, pl // 2],
                                          srcr[:, ry, rx])
                    else:
                        nc.sync.dma_start(xt[:, pl], srcr[:, ry, rx])
                for m in range(mout // P):
                    for r0, nr in row_tiles(hout, hout, 512):
                        npx = nr * hout
                        pt = ps.tile([P, 512], f32, tag="eps")
                        first = True
                        for s in range(4):
                            qy, qx = s // 2, s % 2
                            for k in range(kch):
                                nc.tensor.matmul(
                                    pt[:, :npx], wt[:, s * kch + k, m * P:(m + 1) * P],
                                    xt[:, k, qy + r0:qy + r0 + nr, qx:qx + hout],
                                    start=first, stop=(s == 3 and k == kch - 1))
                                first = False
                        ot = sb.tile([P, 512], f32, tag=f"c{li}o")
                        nc.scalar.activation(ot[:, :npx], pt[:, :npx], AF.Prelu, alpha=alpha[:])
                        nc.sync.dma_start(
                            dst[j][m * P:(m + 1) * P, 1 + r0:1 + r0 + nr, 1:1 + hout],
                            ot[:, :npx].rearrange("c (y x) -> c y x", y=nr).bitcast(f32r))

        # conv4: j outer, shifts outer (PSUM held across shifts), w4 loaded per shift
        for j in range(2):
            xt = plp4.tile([P, 8, 17, 17], f32r, tag="pl3")
            srcr = e3[j][:].rearrange("c (i py) (j2 px) -> c py px i j2", py=2, px=2)
            for pl in range(4):
                ry, rx = pl // 2, pl % 2
                for hh in range(2):
                    nc.sync.dma_start(xt[:, pl * 2 + hh], srcr[hh * P:(hh + 1) * P, ry, rx])
            pts = [ps4.tile([P, 256], f32, tag="e4ps", name=f"e4ps_{j}_{_i}") for _i in range(4)]
            for s in range(4):
                qy, qx = s // 2, s % 2
                wt4 = wp4.tile([P, 8, 512], f32r, tag="w4s")
                nc.sync.dma_start(
                    wt4[:], w4.ap().rearrange("(sk p) m -> p sk m", p=P)[:, s * 8:(s + 1) * 8, :])
                for m in range(4):
                    for k in range(8):
                        nc.tensor.matmul(
                            pts[m][:], wt4[:, k, m * P:(m + 1) * P],
                            xt[:, k, qy:qy + 16, qx:qx + 16],
                            start=(s == 0 and k == 0), stop=(s == 3 and k == 7))
            for m in range(4):
                ot = sb.tile([P, 256], f32, tag="c4o")
                nc.scalar.activation(ot[:], pts[m][:], AF.Prelu, alpha=alpha[:])
                nc.sync.dma_start(enc_out[j, m * P:(m + 1) * P, :], ot[:].bitcast(f32r))

        nc.gpsimd.collective_compute(
            "AllGather", OP.bypass, replica_groups=[list(range(NCORES))],
            ins=[enc_out[:].opt()], outs=[gath_feat[:].opt()])


def build_lstm(nc, tc, wx_l, wh_l, lb_l, gath_feat, gath_h, a2a_recv, pdram, dbg_h, zero_fill):
    NPX = B * 256
    with (
        tc.tile_pool(name="lstm_w", bufs=1) as wp,
        tc.tile_pool(name="lstm_sb", bufs=1) as sb,
        tc.tile_pool(name="lstm_gx", bufs=1) as gxp,
        tc.tile_pool(name="lstm_pl", bufs=1) as plp,
        tc.tile_pool(name="lstm_ps", bufs=4, space="PSUM") as ps,
        tc.tile_pool(name="lstm_dram", bufs=2, space="DRAM") as ldram,
    ):
        h_last = None
        for l in range(KN_LAYERS):
            wx = wp.tile([P, 36, 256], f32r, tag="wx")
            wh = wp.tile([P, 36, 256], f32r, tag="wh")
            nc.sync.dma_start(wx[:], wx_l[l].ap().rearrange("(tk p) m -> p tk m", p=P))
            nc.sync.dma_start(wh[:], wh_l[l].ap().rearrange("(tk p) m -> p tk m", p=P))
            bias = wp.tile([P, 2, 1], f32, tag="lbias")
            nc.sync.dma_start(bias[:], lb_l[l].ap().rearrange("(ch p) o -> p ch o", p=P))

            xp = plp.tile([P, 4, B, 18, 18], f32r, tag="xp")
            hp = plp.tile([P, 4, B, 18, 18], f32r, tag="hp")
            zero_fill(xp[:].rearrange("p a b c d -> p (a b c d)"))
            zero_fill(hp[:].rearrange("p a b c d -> p (a b c d)"))
            gx = gxp.tile([P, 2, KN_STEPS, NPX], f32, tag="gx")

            for t in range(KN_STEPS):
                for k in range(4):
                    for b in range(B):
                        if l == 0:
                            src = gath_feat[b * 4 + t, k * P:(k + 1) * P, :]
                        else:
                            src = gath_h[l - 1][t][k * P:(k + 1) * P, b, :]
                        nc.sync.dma_start(xp[:, k, b, 1:17, 1:17],
                                          src.rearrange("c (y x) -> c y x", y=16))
                for m in range(2):
                    for nh in range(2):
                        pt = ps.tile([P, 512], f32, tag="lps")
                        first = True
                        for tap in range(9):
                            ky, kx = tap // 3, tap % 3
                            for k in range(4):
                                nc.tensor.matmul(
                                    pt[:], wx[:, tap * 4 + k, m * P:(m + 1) * P],
                                    xp[:, k, nh * 2:nh * 2 + 2, ky:ky + 16, kx:kx + 16],
                                    start=first, stop=(tap == 8 and k == 3))
                                first = False
                        nc.scalar.activation(gx[:, m, t, nh * 512:(nh + 1) * 512], pt[:],
                                             AF.Identity, bias=bias[:, m])

            c_t = sb.tile([P, NPX], f32, tag="c_t")
            for t in range(KN_STEPS):
                if t > 0:
                    for k in range(4):
                        for b in range(B):
                            src = gath_h[l][t - 1][k * P:(k + 1) * P, b, :]
                            nc.sync.dma_start(hp[:, k, b, 1:17, 1:17],
                                              src.rearrange("c (y x) -> c y x", y=16))
                    sAB = []
                    for m in range(2):
                        sm = sb.tile([P, NPX], f32, tag=f"s{m}")
                        for nh in range(2):
                            pt = ps.tile([P, 512], f32, tag="lps")
                            first = True
                            for tap in range(9):
                                ky, kx = tap // 3, tap % 3
                                for k in range(4):
                                    nc.tensor.matmul(
                                        pt[:], wh[:, tap * 4 + k, m * P:(m + 1) * P],
                                        hp[:, k, nh * 2:nh * 2 + 2, ky:ky + 16, kx:kx + 16],
                                        start=first, stop=(tap == 8 and k == 3))
                                    first = False
                            nc.vector.tensor_add(sm[:, nh * 512:(nh + 1) * 512], pt[:],
                                                 gx[:, m, t, nh * 512:(nh + 1) * 512])
                        sAB.append(sm)
                    sA, sB_ = sAB
                else:
                    sA = gx[:, 0, 0]
                    sB_ = gx[:, 1, 0]
                nc.scalar.activation(sA[:], sA[:], AF.Sigmoid)        # [sig(i)|sig(f)]
                nc.scalar.activation(sB_[0:64], sB_[0:64], AF.Tanh)    # tanh(g)
                nc.scalar.activation(sB_[64:128], sB_[64:128], AF.Sigmoid)  # sig(o)
                it = sb.tile([64, NPX], f32, tag="it")
                nc.vector.tensor_mul(it[:], sA[0:64], sB_[0:64])
                it_hi = sb.tile([P, NPX], f32, tag="ithi")
                nc.sync.dma_start(it_hi[64:128], it[:])
                if t > 0:
                    nc.vector.tensor_mul(c_t[64:128], sA[64:128], c_t[64:128])
                    nc.vector.tensor_add(c_t[64:128], c_t[64:128], it_hi[64:128])
                else:
                    nc.vector.tensor_copy(c_t[64:128], it_hi[64:128])
                tct = sb.tile([P, NPX], f32, tag="tct")
                nc.scalar.activation(tct[64:128], c_t[64:128], AF.Tanh)
                h_t = sb.tile([P, NPX], f32, tag="h_t")
                nc.vector.tensor_mul(h_t[64:128], sB_[64:128], tct[64:128])
                cc_in = ldram.tile([64, B, 256], f32r, tag="ccin")
                nc.sync.dma_start(cc_in[:],
                                  h_t[64:128].rearrange("c (b px) -> c b px", b=B).bitcast(f32r))
                nc.gpsimd.collective_compute(
                    "AllGather", OP.bypass, replica_groups=[list(range(NCORES))],
                    ins=[cc_in[:].opt()], outs=[gath_h[l][t][:].opt()])
                if l == KN_LAYERS - 1 and t == KN_STEPS - 1:
                    h_last = h_t
        if dbg_h is not None:
            nc.sync.dma_start(dbg_h.ap(), gath_h[KN_LAYERS - 1][KN_STEPS - 1][:].bitcast(f32))

        # AllToAll out_me slab distribution
        send = sb.tile([64, 8, 12 * 16], f32, tag="send")
        nc.vector.memset(send[:], 0.0)
        hl = h_last[64:128].rearrange("c (b y x) -> c b y x", b=B, y=16)
        for d in range(8):
            s_d, cp = d // 2, d % 2
            z0, z1 = (2, 12) if cp == 0 else (0, 10)
            img0 = 8 * cp - 2 + z0
            nc.sync.dma_start(
                send[:, d, z0 * 16:z1 * 16], hl[:, s_d, img0:img0 + (z1 - z0), :])
        a2a_send = ldram.tile([8, 64, 12, 16], f32r, tag="a2asend")
        nc.sync.dma_start(a2a_send[:].rearrange("d c z x -> c d (z x)"),
                          send[:].bitcast(f32r))
        nc.gpsimd.collective_compute(
            "AllToAll", OP.bypass, replica_groups=[list(range(NCORES))],
            ins=[a2a_send[:].opt()], outs=[a2a_recv[:].opt()])


def build_generators(nc, tc, a2a_recv, wg1, bg1, wg2, bg2, wg3, bg3, wg4, bg4,
                     wfin, bfin, wfuse, bfuse, wpad, yflag, topmask,
                     frame_out, flow_out, pred_out, zero128, zero_fill):
    with (
        tc.tile_pool(name="gen_w", bufs=1) as wp,
        tc.tile_pool(name="gen_w1", bufs=2) as wp1,
        tc.tile_pool(name="gen_sb", bufs=1) as sb,
        tc.tile_pool(name="gen_pl", bufs=1) as plp,
        tc.tile_pool(name="gen_ps", bufs=4, space="PSUM") as ps,
        tc.tile_pool(name="gen_dram", bufs=1, space="DRAM") as gdram,
    ):
        # TRUE-interleaved images in DRAM (+1 col pad to avoid AP merging).
        def ibuf(nm, cch, rr, cc):
            return [gdram.tile([cch, rr, cc + 1], f32r, tag=f"{nm}_{g}", name=f"{nm}_{g}")
                    for g in range(2)]
        s1 = ibuf("s1", 512, 32, 32)
        s2 = ibuf("s2", 256, 36, 64)
        s3 = ibuf("s3", 128, 68, 128)
        s4 = ibuf("s4", 64, 132, 256)
        warp_dram = gdram.tile([3, 128, 256], f32r, tag="warp_dram")

        def store_phase(dstbuf, cs0, csz, py, px, rt0, rn, win, stg):
            # per-row DMAs: dst [c, win step-2 cols], src [c, win]
            for r in range(rn):
                nc.sync.dma_start(
                    dstbuf[cs0:cs0 + csz, 2 * (rt0 + r) + py, px:2 * win - 1 + px:2],
                    stg[:csz, r, :win].bitcast(f32r))

        # ---- L1 merged (M = 1024) ----
        zt = plp.tile([P, 4, 17, 17], f32r, tag="zt")
        zero_fill(zt[:].rearrange("p a b c -> p (a b c)"))
        for k in range(4):
            for half in range(2):
                nc.sync.dma_start(zt[half * 64:half * 64 + 64, k, 0:12, 0:16],
                                  a2a_recv[k * 2 + half])
        b1b = wp.tile([P, 8, 1], f32, tag="bias1")
        nc.sync.dma_start(b1b[:], bg1.ap().rearrange("(mm p) o -> p mm o", p=P))
        for py in (0, 1):
            for px in (0, 1):
                taps = [(ky, kx, dy, dx) for (a_, b_, ky, kx, dy, dx) in DEC_TAPS
                        if a_ == py and b_ == px]
                for mg in range(2):
                    pts = [ps.tile([P, 16, 16], f32, tag="gps", name=f"l1p_{py}{px}{mg}{_i}")
                           for _i in range(4)]
                    for ti, (ky, kx, dy, dx) in enumerate(taps):
                        tap_idx = DEC_TAPS.index((py, px, ky, kx, dy, dx))
                        wt = wp1.tile([P, 4, 512], f32r, tag="wg1t")
                        nc.sync.dma_start(
                            wt[:],
                            wg1.ap().rearrange("(tk p) m -> p tk m", p=P)[
                                :, tap_idx * 4:(tap_idx + 1) * 4, mg * 512:(mg + 1) * 512])
                        for k in range(4):
                            for mi in range(4):
                                nc.tensor.matmul(
                                    pts[mi][:].rearrange("c y x -> c (y x)"),
                                    wt[:, k, mi * P:(mi + 1) * P],
                                    zt[:, k, dy:dy + 16, dx:dx + 16],
                                    start=(ti == 0 and k == 0),
                                    stop=(ti == len(taps) - 1 and k == 3))
                    for mi in range(4):
                        m = mg * 4 + mi
                        g, mm_ = m // 4, m % 4
                        stg = sb.tile([P, 16, 17], f32, tag="l1stg")
                        nc.scalar.activation(stg[:, :, :16], pts[mi][:], AF.Relu,
                                             bias=b1b[:, m])
                        store_phase(s1[g], mm_ * P, P, py, px, 0, 16, 16, stg)

        # ---- L2..L4 per gen ----
        for (wd, bd, srcs, r0_, r1_, dsts, cin, win, mout) in (
                (wg2, bg2, s1, 3, 21, s2, 512, 32, 256),
                (wg3, bg3, s2, 1, 35, s3, 256, 64, 128),
                (wg4, bg4, s3, 1, 67, s4, 128, 128, 64)):
            kch = cin // P
            nrows = r1_ - r0_
            msize = min(P, mout)
            mchunks = mout // msize
            rn_max = max(1, 512 // win)
            for g in range(2):
                wt = wp.tile([P, 9 * kch, mout], f32r, tag=f"wg_{cin}")
                nc.sync.dma_start(wt[:], wd[g].ap().rearrange("(tk p) m -> p tk m", p=P))
                bt = wp.tile([msize, mchunks, 1], f32, tag=f"bg_{cin}")
                nc.sync.dma_start(bt[:], bd[g].ap().rearrange("(mm p) o -> p mm o", p=msize))
                xt = plp.tile([P, kch, nrows + 1, win + 1], f32r, tag=f"gpl_{cin}")
                zero_fill(xt[:].rearrange("p a b c -> p (a b c)"))
                for k in range(kch):
                    nc.sync.dma_start(xt[:, k, 0:nrows, 0:win],
                                      srcs[g][k * P:(k + 1) * P, r0_:r1_, :win])
                for py in (0, 1):
                    for px in (0, 1):
                        taps = [(ky, kx, dy, dx) for (a_, b_, ky, kx, dy, dx) in DEC_TAPS
                                if a_ == py and b_ == px]
                        for m in range(mchunks):
                            for rt0, rn in row_tiles(nrows, win, 512):
                                npx = rn * win
                                pt = ps.tile([P, rn_max, win], f32, tag="gps")
                                first = True
                                for ti, (ky, kx, dy, dx) in enumerate(taps):
                                    tap_idx = DEC_TAPS.index((py, px, ky, kx, dy, dx))
                                    for k in range(kch):
                                        nc.tensor.matmul(
                                            pt[:msize, :rn].rearrange("c y x -> c (y x)"),
                                            wt[:, tap_idx * kch + k, m * msize:(m + 1) * msize],
                                            xt[:, k, dy + rt0:dy + rt0 + rn, dx:dx + win],
                                            start=first,
                                            stop=(ti == len(taps) - 1 and k == kch - 1))
                                        first = False
                                stg = sb.tile([P, rn_max, win + 1], f32, tag="gstg")
                                nc.scalar.activation(stg[:msize, :rn, :win],
                                                     pt[:msize, :rn], AF.Relu, bias=bt[:, m])
                                store_phase(dsts[g], m * msize, msize, py, px,
                                            rt0, rn, win, stg)

        # zero image row -1 on top-slab cores (final-conv zero padding)
        for g in range(2):
            mrow = sb.tile([64, 256], f32r, tag="maskrow")
            mmask = sb.tile([64, 256], f32, tag="maskval")
            nc.sync.dma_start(mrow[:], s4[g][:, 1, :256])
            nc.sync.dma_start(mmask[:], topmask.ap().bitcast(f32))
            nc.vector.tensor_mul(mrow[:].bitcast(f32), mrow[:].bitcast(f32), mmask[:])
            nc.sync.dma_start(s4[g][:, 1, :256], mrow[:])

        # ---- final conv (frame||flow K-stacked, M=5) ----
        wfin_t = wp.tile([P, 9, 5], f32r, tag="wfin")
        nc.sync.dma_start(wfin_t[:], wfin.ap().rearrange("(t p) m -> p t m", p=P))
        bfin_t = wp.tile([5, 1], f32, tag="bfin")
        nc.sync.dma_start(bfin_t[:], bfin.ap())
        fin_pl = plp.tile([P, 12, 258], f32r, tag="fin_pl")
        zero_fill(fin_pl[:].rearrange("p a b -> p (a b)"))
        for rt0 in range(0, 128, 10):
            rn = min(10, 128 - rt0)
            for g in range(2):
                nc.sync.dma_start(fin_pl[g * 64:g * 64 + 64, 0:rn + 2, 1:257],
                                  s4[g][:, 1 + rt0:1 + rt0 + rn + 2, :256])
            for st0, sn in row_tiles(rn, 256, 512):
                npx = sn * 256
                pt = ps.tile([P, 512], f32, tag="gps")
                first = True
                for tap in range(9):
                    ky, kx = tap // 3, tap % 3
                    nc.tensor.matmul(pt[:5, :npx], wfin_t[:, tap, :],
                                     fin_pl[:, st0 + ky:st0 + ky + sn, kx:kx + 256],
                                     start=first, stop=(tap == 8))
                    first = False
                sig = sb.tile([5, 512], f32, tag="finsig")
                nc.scalar.activation(sig[:, :npx], pt[:5, :npx], AF.Sigmoid, bias=bfin_t[:])
                tnh = sb.tile([5, 512], f32, tag="fintanh")
                nc.scalar.activation(tnh[:, :npx], pt[:5, :npx], AF.Tanh, bias=bfin_t[:])
                rr = rt0 + st0
                nc.sync.dma_start(frame_out.ap()[:, rr:rr + sn, :],
                                  sig[0:3, :npx].rearrange("c (y x) -> c y x", y=sn))
                nc.sync.dma_start(flow_out.ap()[:, rr:rr + sn, :],
                                  tnh[3:5, :npx].rearrange("c (y x) -> c y x", y=sn))

        build_warp(nc, sb, wp, wpad, yflag, flow_out, warp_dram)

        # fuse 1x1 conv + sigmoid
        wfu = wp.tile([6, 3], f32r, tag="wfu")
        nc.sync.dma_start(wfu[:], wfuse.ap())
        bfu = wp.tile([3, 1], f32, tag="bfu")
        nc.sync.dma_start(bfu[:], bfuse.ap())
        frame_flat = frame_out.ap().rearrange("c y x -> c (y x)")
        warp_flat = warp_dram[:].rearrange("c y x -> c (y x)")
        for n0 in range(0, 128 * 256, 512):
            ft = sb.tile([6, 512], f32r, tag="fusein")
            nc.sync.dma_start(ft[0:3, :], frame_flat[:, n0:n0 + 512].bitcast(f32r))
            nc.sync.dma_start(ft[3:6, :], warp_flat[:, n0:n0 + 512])
            pt = ps.tile([P, 512], f32, tag="gps")
            nc.tensor.matmul(pt[:3], wfu[:], ft[:], start=True, stop=True)
            ot = sb.tile([3, 512], f32, tag="fuseo")
            nc.scalar.activation(ot[:], pt[:3], AF.Sigmoid, bias=bfu[:])
            nc.sync.dma_start(pred_out.ap().rearrange("c y x -> c (y x)")[:, n0:n0 + 512], ot[:])


def build_warp(nc, sb, wp, wpad, yflag, flow_out, warp_dram):
    yfl = wp.tile([1, 1], f32, tag="yfl")
    nc.sync.dma_start(yfl[:], yflag.ap())
    for chunk in range(2):
        r0 = chunk * 64
        fx = sb.tile([64, 256], f32, tag="wfx")
        fy = sb.tile([64, 256], f32, tag="wfy")
        nc.sync.dma_start(fx[:], flow_out.ap()[0, r0:r0 + 64, :])
        nc.sync.dma_start(fy[:], flow_out.ap()[1, r0:r0 + 64, :])
        planes = {}
        for nm, f_ in (("x", fx), ("y", fy)):
            pp = sb.tile([64, 256], f32, tag=f"w{nm}p")
            mm_ = sb.tile([64, 256], f32, tag=f"w{nm}m")
            zz = sb.tile([64, 256], f32, tag=f"w{nm}0")
            nc.scalar.activation(pp[:], f_[:], AF.Relu)
            nc.scalar.activation(mm_[:], f_[:], AF.Relu, scale=-1.0)
            nc.scalar.activation(zz[:], f_[:], AF.Abs)
            nc.vector.tensor_scalar(zz[:], zz[:], -1.0, 1.0, OP.mult, OP.add)
            planes[nm] = (pp, mm_, zz)
        wxp, wxm, wx0 = planes["x"]
        wyp, wym, wy0 = planes["y"]
        # col-0 / row-0 correction factors
        tcol = sb.tile([64, 1], f32, tag="tcol")
        mneg = sb.tile([64, 1], f32, tag="mneg")
        nc.vector.tensor_scalar(mneg[:], fx[:, 0:1], 0.0, None, OP.is_lt)
        nc.vector.tensor_scalar(tcol[:], fx[:, 0:1], 1.0, None, OP.add)
        nc.vector.tensor_mul(tcol[:], tcol[:], mneg[:])
        trow = sb.tile([1, 256], f32, tag="trow")
        mrow = sb.tile([1, 256], f32, tag="mrow")
        nc.vector.tensor_scalar(mrow[:], fy[0:1, :], 0.0, None, OP.is_lt)
        nc.vector.tensor_scalar(trow[:], fy[0:1, :], 1.0, None, OP.add)
        nc.vector.tensor_mul(trow[:], trow[:], mrow[:])
        nc.vector.tensor_scalar_mul(trow[:], trow[:], yfl[:1])

        for ch in range(3):
            wrow = sb.tile([66, 258], f32, tag="wrow")
            nc.sync.dma_start(wrow[:], wpad.ap()[ch, r0:r0 + 66, :])
            shifted = []
            for si, nm in ((0, "im"), (1, "i0"), (2, "ip")):
                tl = sb.tile([64, 258], f32, tag=nm)
                nc.sync.dma_start(tl[:], wrow[si:si + 64, :])
                shifted.append(tl)
            X = []
            for img, nm in zip(shifted, ("xm", "x0", "xp")):
                xi = sb.tile([64, 256], f32, tag=f"X{nm}")
                tmp = sb.tile([64, 256], f32, tag=f"Xt{nm}")
                nc.vector.tensor_mul(xi[:], wx0[:], img[:, 1:257])
                nc.vector.tensor_mul(tmp[:], wxp[:], img[:, 2:258])
                nc.vector.tensor_add(xi[:], xi[:], tmp[:])
                nc.vector.tensor_mul(tmp[:], wxm[:], img[:, 0:256])
                nc.vector.tensor_add(xi[:], xi[:], tmp[:])
                d01 = sb.tile([64, 1], f32, tag=f"d{nm}")
                nc.vector.tensor_tensor(d01[:], img[:, 2:3], img[:, 1:2], OP.subtract)
                nc.vector.tensor_mul(d01[:], d01[:], tcol[:])
                nc.vector.tensor_add(xi[:, 0:1], xi[:, 0:1], d01[:])
                X.append(xi)
            Xm, X0, Xp = X
            v = sb.tile([64, 256], f32, tag="wv")
            tmp2 = sb.tile([64, 256], f32, tag="wvt")
            nc.vector.tensor_mul(v[:], wy0[:], X0[:])
            nc.vector.tensor_mul(tmp2[:], wyp[:], Xp[:])
            nc.vector.tensor_add(v[:], v[:], tmp2[:])
            nc.vector.tensor_mul(tmp2[:], wym[:], Xm[:])
            nc.vector.tensor_add(v[:], v[:], tmp2[:])
            if chunk == 0:
                dr = sb.tile([1, 256], f32, tag="dr")
                nc.vector.tensor_tensor(dr[:], Xp[0:1, :], X0[0:1, :], OP.subtract)
                nc.vector.tensor_mul(dr[:], dr[:], trow[:])
                nc.vector.tensor_add(v[0:1, :], v[0:1, :], dr[:])
            nc.sync.dma_start(warp_dram[ch, r0:r0 + 64, :], v[:].bitcast(f32r))


# ==================================================================== host

_CACHE = {}


def _prep_weights(cnn_ws, lstm_ws, lstm_bs, gframe_ws, gframe_bs, gflow_ws,
                  gflow_bs, fuse_w, fuse_b):
    f = np.float32
    out = {}
    w1 = np.asarray(cnn_ws[0], f)          # (64, 3, 4, 4)
    out["w1"] = np.ascontiguousarray(w1.transpose(2, 3, 1, 0).reshape(48, 64))
    for li, nm in ((1, "w2"), (2, "w3"), (3, "w4")):
        w = np.asarray(cnn_ws[li], f)      # (Co, Ci, 4, 4)
        Co, Ci = w.shape[:2]
        shifts = []
        for qy in range(2):
            for qx in range(2):
                blocks = []
                for ry in range(2):
                    for rx in range(2):
                        blocks.append(w[:, :, 2 * qy + ry, 2 * qx + rx].T)  # (Ci, Co)
                shifts.append(np.concatenate(blocks, 0))                    # (4Ci, Co)
        out[nm] = np.ascontiguousarray(np.stack(shifts, 0).reshape(4 * 4 * Ci // P * P, Co))

    # per-core LSTM slices are produced later (need core index)
    out["_lstm_w"] = [np.asarray(w, f) for w in lstm_ws]
    out["_lstm_b"] = [np.asarray(b, f) for b in lstm_bs]

    def gen_layer(w):                       # deconv (Co, Ci, 3, 3) -> [9*KCH*128, Co]
        w = np.asarray(w, f)
        Co, Ci = w.shape[:2]
        taps = []
        for (py, ky, dy) in DEC_YT:
            for (px, kx, dx) in DEC_YT:
                taps.append(w[:, :, ky, kx].T)   # (Ci, Co)
        return np.ascontiguousarray(np.stack(taps, 0).reshape(9 * Ci, Co))

    wf, wl = [np.asarray(w, f) for w in gframe_ws], [np.asarray(w, f) for w in gflow_ws]
    bf = [np.asarray(b, f) for b in gframe_bs]
    bl = [np.asarray(b, f) for b in gflow_bs]
    out["wg1"] = np.concatenate([
        gen_layer(wf[0]).reshape(9, 512, 512), gen_layer(wl[0]).reshape(9, 512, 512)],
        axis=2).reshape(9 * 512, 1024)
    out["bg1"] = np.concatenate([bf[0], bl[0]])[:, None]
    for i, nm in ((1, "wg2"), (2, "wg3"), (3, "wg4")):
        out[f"{nm}_0"] = gen_layer(wf[i])
        out[f"{nm}_1"] = gen_layer(wl[i])
        out[f"b{nm[1:]}_0"] = bf[i][:, None]
        out[f"b{nm[1:]}_1"] = bl[i][:, None]
    # final conv: K = 64frame || 64flow stacked, M = 5
    wff, wlf = np.asarray(gframe_ws[4], f), np.asarray(gflow_ws[4], f)  # (3,64,3,3),(2,64,3,3)
    taps = []
    for ky in range(3):
        for kx in range(3):
            blk = np.zeros((128, 5), f)
            blk[0:64, 0:3] = wff[:, :, ky, kx].T
            blk[64:128, 3:5] = wlf[:, :, ky, kx].T
            taps.append(blk)
    out["wfin"] = np.concatenate(taps, 0)
    out["bfin"] = np.concatenate([np.asarray(gframe_bs[4], f),
                                  np.asarray(gflow_bs[4], f)])[:, None]
    out["wfuse"] = np.ascontiguousarray(np.asarray(fuse_w, f)[:, :, 0, 0].T)
    out["bfuse"] = np.asarray(fuse_b, f)[:, None]
    return out


def _lstm_core_slices(wl, bl, c):
    rows = []
    for gate in range(4):
        rows.extend(range(gate * 512 + 64 * c, gate * 512 + 64 * c + 64))
    # chunk order: [i|f] then [g|o]
    rows = np.array(rows[:128] + rows[128:], np.int64)
    ws = wl[rows]                     # (256, 1024, 3, 3)
    wx = ws[:, :512].transpose(2, 3, 1, 0).reshape(9, 512, 256).reshape(9 * 512, 256)
    whp = ws[:, 512:].transpose(2, 3, 1, 0).reshape(9, 512, 256).reshape(9 * 512, 256)
    bb = bl[rows][:, None]
    return (np.ascontiguousarray(wx), np.ascontiguousarray(whp),
            np.ascontiguousarray(bb))


def _im2col_conv1(frames):
    # frames: (2, 3, 256, 256) -> (2, 48, 128*128), tap-major rows (t*3+c)
    fpad = np.pad(frames, ((0, 0), (0, 0), (1, 1), (1, 1)))
    taps = []
    for dy in range(4):
        for dx in range(4):
            taps.append(fpad[:, :, dy:dy + 256:2, dx:dx + 256:2])
    arr = np.stack(taps, 1)  # (2, 16, 3, 128, 128)
    return np.ascontiguousarray(arr.reshape(2, 48, 128 * 128))


def kernel(x, cnn_ws, lstm_ws, lstm_bs, gframe_ws, gframe_bs, gflow_ws,
           gflow_bs, fuse_w, fuse_b):
    x = np.asarray(x, np.float32)
    wd = _prep_weights(cnn_ws, lstm_ws, lstm_bs, gframe_ws, gframe_bs,
                       gflow_ws, gflow_bs, fuse_w, fuse_b)

    frames = x.transpose(0, 2, 1, 3, 4).reshape(B * T, C, H, W)
    in_maps = []
    shared = {k: v for k, v in wd.items() if not k.startswith("_")}
    for cidx in range(NCORES):
        m = dict(shared)
        m["enc_in"] = _im2col_conv1(frames[2 * cidx:2 * cidx + 2])
        for l in range(3):
            wx, wh_, bb = _lstm_core_slices(wd["_lstm_w"][l], wd["_lstm_b"][l], cidx)
            m[f"wx{l}"], m[f"wh{l}"], m[f"lb{l}"] = wx, wh_, bb
        s, cp = cidx // 2, cidx % 2
        prev = x[s, :, -1]                         # (3, 256, 256)
        prow = np.pad(prev, ((0, 0), (1, 1), (1, 1)), mode="edge")  # (3,258,258)
        m["wpad"] = np.ascontiguousarray(prow[:, 128 * cp:128 * cp + 130, :])
        m["yflag"] = np.array([[1.0 if cp == 0 else 0.0]], np.float32)
        m["topmask"] = np.full((64, 256), 0.0 if cp == 0 else 1.0, np.float32)
        in_maps.append(m)

    if "nc" not in _CACHE:
        _CACHE["nc"] = build_program()
    nc = _CACHE["nc"]

    from concourse.bass_utils import run_bass_kernel_spmd
    res = run_bass_kernel_spmd(nc, in_maps, core_ids=list(range(NCORES)))
    results = res.results

    frame_pred = np.zeros((B, 3, H, W), np.float32)
    flow_pred = np.zeros((B, 2, H, W), np.float32)
    prediction = np.zeros((B, 3, H, W), np.float32)
    for cidx in range(NCORES):
        s, cp = cidx // 2, cidx % 2
        sl = slice(128 * cp, 128 * cp + 128)
        frame_pred[s, :, sl] = results[cidx]["frame_out"]
        flow_pred[s, :, sl] = results[cidx]["flow_out"]
        prediction[s, :, sl] = results[cidx]["pred_out"]
    if STAGE == "enc":
        return results[0]["dbg_feat"]
    if STAGE == "lstm":
        return results[0]["dbg_h"]
    return frame_pred, flow_pred, prediction


# revision 34
# speedup vs baseline: 93.4245x; 90.9044x over previous
"""DualMotionGAN forward on 8 Trainium2 NeuronCores (Bass/Tile, float32r matmuls).

Distribution: encoder data-parallel over the 16 frames (2 per core, stride-2
convs phase-decomposed into full-K tap matmuls); 3-layer ConvLSTM 8-way
gate-channel split (each core computes 64 channels of each gate for all
samples; h is AllGathered every step); an AllToAll then hands each core the
(sample, row-half) slab of out_me it needs, so the two deconv generators,
the bilinear warp (elementwise: flow=tanh in (-1,1) touches only the 3x3
neighborhood), and the fuse conv run spatially split with no further
communication. All per-core variation enters via host-sliced inputs --
the device program is identical on every core.
"""
import os
import numpy as np

import concourse.bass as bass
from concourse import bacc
import concourse.mybir as mybir
import concourse.tile as tile

f32r = mybir.dt.float32r
f32 = mybir.dt.float32
AF = mybir.ActivationFunctionType
OP = mybir.AluOpType
P = 128
NCORES = 8
B, C, T, H, W = 4, 3, 4, 256, 256

# deconv tap table: (phase, kernel index, input shift)
DEC_YT = [(0, 1, 0), (1, 0, 0), (1, 2, 1)]
DEC_TAPS = [(py, px, ky, kx, dy, dx)
            for (py, ky, dy) in DEC_YT for (px, kx, dx) in DEC_YT]

STAGE = os.environ.get("KSTAGE", "full")
KN_LAYERS = int(os.environ.get("KN_LAYERS", "3"))
KN_STEPS = int(os.environ.get("KN_STEPS", "4"))


def row_tiles(nrows, width, maxn=512):
    rpt = max(1, maxn // width)
    out = []
    r = 0
    while r < nrows:
        n = min(rpt, nrows - r)
        out.append((r, n))
        r += n
    return out


# =================================================================== device

def build_program():
    nc = bacc.Bacc()

    enc_in = nc.dram_tensor("enc_in", [2, 48, 128 * 128], f32r, kind="ExternalInput")
    w1 = nc.dram_tensor("w1", [48, 64], f32r, kind="ExternalInput")
    w2 = nc.dram_tensor("w2", [4 * 2 * P, 128], f32r, kind="ExternalInput")
    w3 = nc.dram_tensor("w3", [4 * 4 * P, 256], f32r, kind="ExternalInput")
    w4 = nc.dram_tensor("w4", [4 * 8 * P, 512], f32r, kind="ExternalInput")
    wx_l = [nc.dram_tensor(f"wx{l}", [9 * 4 * P, 256], f32r, kind="ExternalInput") for l in range(3)]
    wh_l = [nc.dram_tensor(f"wh{l}", [9 * 4 * P, 256], f32r, kind="ExternalInput") for l in range(3)]
    lb_l = [nc.dram_tensor(f"lb{l}", [2 * P, 1], f32, kind="ExternalInput") for l in range(3)]
    wg1 = nc.dram_tensor("wg1", [9 * 4 * P, 1024], f32r, kind="ExternalInput")
    bg1 = nc.dram_tensor("bg1", [8 * P, 1], f32, kind="ExternalInput")
    wg2 = [nc.dram_tensor(f"wg2_{g}", [9 * 4 * P, 256], f32r, kind="ExternalInput") for g in range(2)]
    bg2 = [nc.dram_tensor(f"bg2_{g}", [2 * P, 1], f32, kind="ExternalInput") for g in range(2)]
    wg3 = [nc.dram_tensor(f"wg3_{g}", [9 * 2 * P, 128], f32r, kind="ExternalInput") for g in range(2)]
    bg3 = [nc.dram_tensor(f"bg3_{g}", [P, 1], f32, kind="ExternalInput") for g in range(2)]
    wg4 = [nc.dram_tensor(f"wg4_{g}", [9 * P, 64], f32r, kind="ExternalInput") for g in range(2)]
    bg4 = [nc.dram_tensor(f"bg4_{g}", [64, 1], f32, kind="ExternalInput") for g in range(2)]
    wfin = nc.dram_tensor("wfin", [9 * P, 5], f32r, kind="ExternalInput")
    bfin = nc.dram_tensor("bfin", [5, 1], f32, kind="ExternalInput")
    wfuse = nc.dram_tensor("wfuse", [6, 3], f32r, kind="ExternalInput")
    bfuse = nc.dram_tensor("bfuse", [3, 1], f32, kind="ExternalInput")
    wpad = nc.dram_tensor("wpad", [3, 130, 258], f32, kind="ExternalInput")
    topmask = nc.dram_tensor("topmask", [64, 256], f32r, kind="ExternalInput")
    yflag = nc.dram_tensor("yflag", [1, 1], f32, kind="ExternalInput")

    frame_out = nc.dram_tensor("frame_out", [3, 128, 256], f32, kind="ExternalOutput")
    flow_out = nc.dram_tensor("flow_out", [2, 128, 256], f32, kind="ExternalOutput")
    pred_out = nc.dram_tensor("pred_out", [3, 128, 256], f32, kind="ExternalOutput")
    dbg_feat = (nc.dram_tensor("dbg_feat", [16, 512, 256], f32, kind="ExternalOutput")
                if STAGE == "enc" else None)
    dbg_h = (nc.dram_tensor("dbg_h", [512, B, 256], f32, kind="ExternalOutput")
             if STAGE == "lstm" else None)

    with tile.TileContext(nc) as tc:
        with (
            tc.tile_pool(name="persist_dram", bufs=1, space="DRAM") as pdram,
            tc.tile_pool(name="zp", bufs=1) as zpool,
        ):
            zero128 = zpool.tile([P, 1024], f32)
            nc.vector.memset(zero128[:], 0.0)
            zdram = pdram.tile([P, 1024], f32r, tag="zdram")
            nc.sync.dma_start(zdram[:], zero128[:].bitcast(f32r))

            def zero_fill(ap):
                flat = ap  # expects [128, N] contiguous view
                n = flat.shape[-1]
                for n0 in range(0, n, 1024):
                    w_ = min(1024, n - n0)
                    nc.sync.dma_start(flat[:, n0:n0 + w_], zdram[:, :w_])

            gath_feat = pdram.tile([16, 512, 256], f32r)
            gath_h = [[pdram.tile([512, B, 256], f32r, tag=f"gh{l}_{t}", name=f"gh{l}_{t}")
                       for t in range(T)] for l in range(3)]
            a2a_recv = pdram.tile([8, 64, 12, 16], f32r)

            build_encoder(nc, tc, enc_in, w1, w2, w3, w4, gath_feat, pdram, zero128)
            if STAGE == "enc":
                nc.sync.dma_start(dbg_feat.ap(), gath_feat[:].bitcast(f32))
            if STAGE != "enc":
                build_lstm(nc, tc, wx_l, wh_l, lb_l, gath_feat, gath_h, a2a_recv,
                           pdram, dbg_h, zero_fill)
            if STAGE in ("gen", "full"):
                build_generators(nc, tc, a2a_recv, wg1, bg1, wg2, bg2, wg3, bg3,
                                 wg4, bg4, wfin, bfin, wfuse, bfuse, wpad, yflag,
                                 topmask, frame_out, flow_out, pred_out, zero128, zero_fill)
            else:
                for t_ in (frame_out, flow_out, pred_out):
                    tv = t_.ap().rearrange("c y x -> c (y x)")
                    for n0 in range(0, 128 * 256, 1024):
                        nc.sync.dma_start(tv[:, n0:n0 + 1024], zero128[:t_.shape[0], :])
    nc.compile()
    return nc


def build_encoder(nc, tc, enc_in, w1, w2, w3, w4, gath_feat, pdram, zero128):
    with (
        tc.tile_pool(name="enc_w", bufs=1) as wp,
        tc.tile_pool(name="enc_w4", bufs=2) as wp4,
        tc.tile_pool(name="enc_sb", bufs=3) as sb,
        tc.tile_pool(name="enc_pl", bufs=1) as plp,
        tc.tile_pool(name="enc_pl4", bufs=2) as plp4,
        tc.tile_pool(name="enc_ps", bufs=4, space="PSUM") as ps,
        tc.tile_pool(name="enc_ps4", bufs=4, space="PSUM") as ps4,
        tc.tile_pool(name="enc_dram", bufs=1, space="DRAM") as edram,
    ):
        alpha = wp.tile([P, 1], f32, tag="alpha")
        nc.vector.memset(alpha[:], 0.2)
        e1 = [edram.tile([64, 130, 130], f32r, tag=f"e1_{j}", name=f"e1_{j}") for j in range(2)]
        e2 = [edram.tile([128, 66, 66], f32r, tag=f"e2_{j}", name=f"e2_{j}") for j in range(2)]
        e3 = [edram.tile([256, 34, 34], f32r, tag=f"e3_{j}", name=f"e3_{j}") for j in range(2)]
        enc_out = pdram.tile([2, 512, 256], f32r)
        for j in range(2):
            for buf, cch, hp_ in ((e1[j], 64, 130), (e2[j], 128, 66), (e3[j], 256, 34)):
                for cc0 in range(0, cch, P):
                    cn = min(P, cch - cc0)
                    z = zero128[:cn, :1]
                    nc.sync.dma_start(buf[cc0:cc0 + cn, 0, :].bitcast(f32), zero128[:cn, :hp_])
                    nc.sync.dma_start(buf[cc0:cc0 + cn, hp_ - 1, :].bitcast(f32), zero128[:cn, :hp_])
                    nc.sync.dma_start(buf[cc0:cc0 + cn, 1:hp_ - 1, 0].bitcast(f32), zero128[:cn, :hp_ - 2])
                    nc.sync.dma_start(buf[cc0:cc0 + cn, 1:hp_ - 1, hp_ - 1].bitcast(f32), zero128[:cn, :hp_ - 2])

        # conv1 (K=48 host-im2col)
        w1t = wp.tile([48, 64], f32r, tag="w1")
        nc.sync.dma_start(w1t[:], w1.ap())
        for j in range(2):
            imt = plp.tile([48, 128 * 128], f32r, tag="im2col")
            nc.sync.dma_start(imt[:], enc_in.ap()[j])
            for r0, nr in row_tiles(128, 128, 512):
                pt = ps.tile([64, 512], f32, tag="eps")
                nc.tensor.matmul(pt[:, :nr * 128], w1t[:], imt[:, r0 * 128:(r0 + nr) * 128],
                                 start=True, stop=True)
                ot = sb.tile([64, 512], f32, tag="c1o")
                nc.scalar.activation(ot[:, :nr * 128], pt[:, :nr * 128], AF.Prelu, alpha=alpha[:64])
                nc.sync.dma_start(e1[j][:, 1 + r0:1 + r0 + nr, 1:129],
                                  ot[:, :nr * 128].rearrange("c (y x) -> c y x", y=nr).bitcast(f32r))

        # conv2 / conv3 (shifts inner, weights fully resident)
        for li, (wdram, src, dst, cin, hin2, hout, mout) in enumerate((
                (w2, e1, e2, 64, 65, 64, 128),
                (w3, e2, e3, 128, 33, 32, 256))):
            kch = 4 * cin // P
            wt = wp.tile([P, 4 * kch, mout], f32r, tag=f"wenc{li}")
            nc.sync.dma_start(wt[:], wdram.ap().rearrange("(sk p) m -> p sk m", p=P))
            for j in range(2):
                xt = plp.tile([P, kch, hin2, hin2], f32r, tag=f"pl{li}")
                srcr = src[j][:].rearrange("c (i py) (j2 px) -> c py px i j2", py=2, px=2)
                for pl in range(4):
                    ry, rx = pl // 2, pl % 2
                    if cin == 64:
                        nc.sync.dma_start(xt[(pl % 2) * 64:(pl % 2) * 64 + 64, pl // 2],
                                          srcr[:, ry, rx])
                    else:
                        nc.sync.dma_start(xt[:, pl], srcr[:, ry, rx])
                for m in range(mout // P):
                    for r0, nr in row_tiles(hout, hout, 512):
                        npx = nr * hout
                        pt = ps.tile([P, 512], f32, tag="eps")
                        first = True
                        for s in range(4):
                            qy, qx = s // 2, s % 2
                            for k in range(kch):
                                nc.tensor.matmul(
                                    pt[:, :npx], wt[:, s * kch + k, m * P:(m + 1) * P],
                                    xt[:, k, qy + r0:qy + r0 + nr, qx:qx + hout],
                                    start=first, stop=(s == 3 and k == kch - 1))
                                first = False
                        ot = sb.tile([P, 512], f32, tag=f"c{li}o")
                        nc.scalar.activation(ot[:, :npx], pt[:, :npx], AF.Prelu, alpha=alpha[:])
                        nc.sync.dma_start(
                            dst[j][m * P:(m + 1) * P, 1 + r0:1 + r0 + nr, 1:1 + hout],
                            ot[:, :npx].rearrange("c (y x) -> c y x", y=nr).bitcast(f32r))

        # conv4: j outer, shifts outer (PSUM held across shifts), w4 loaded per shift
        for j in range(2):
            xt = plp4.tile([P, 8, 17, 17], f32r, tag="pl3")
            srcr = e3[j][:].rearrange("c (i py) (j2 px) -> c py px i j2", py=2, px=2)
            for pl in range(4):
                ry, rx = pl // 2, pl % 2
                for hh in range(2):
                    nc.sync.dma_start(xt[:, pl * 2 + hh], srcr[hh * P:(hh + 1) * P, ry, rx])
            pts = [ps4.tile([P, 256], f32, tag="e4ps", name=f"e4ps_{j}_{_i}") for _i in range(4)]
            for s in range(4):
                qy, qx = s // 2, s % 2
                wt4 = wp4.tile([P, 8, 512], f32r, tag="w4s")
                nc.sync.dma_start(
                    wt4[:], w4.ap().rearrange("(sk p) m -> p sk m", p=P)[:, s * 8:(s + 1) * 8, :])
                for m in range(4):
                    for k in range(8):
                        nc.tensor.matmul(
                            pts[m][:], wt4[:, k, m * P:(m + 1) * P],
                            xt[:, k, qy:qy + 16, qx:qx + 16],
                            start=(s == 0 and k == 0), stop=(s == 3 and k == 7))
            for m in range(4):
                ot = sb.tile([P, 256], f32, tag="c4o")
                nc.scalar.activation(ot[:], pts[m][:], AF.Prelu, alpha=alpha[:])
                nc.sync.dma_start(enc_out[j, m * P:(m + 1) * P, :], ot[:].bitcast(f32r))

        nc.gpsimd.collective_compute(
            "AllGather", OP.bypass, replica_groups=[list(range(NCORES))],
            ins=[enc_out[:].opt()], outs=[gath_feat[:].opt()])


def build_lstm(nc, tc, wx_l, wh_l, lb_l, gath_feat, gath_h, a2a_recv, pdram, dbg_h, zero_fill):
    NPX = B * 256
    with (
        tc.tile_pool(name="lstm_w", bufs=1) as wp,
        tc.tile_pool(name="lstm_sb", bufs=1) as sb,
        tc.tile_pool(name="lstm_gx", bufs=1) as gxp,
        tc.tile_pool(name="lstm_pl", bufs=1) as plp,
        tc.tile_pool(name="lstm_ps", bufs=4, space="PSUM") as ps,
        tc.tile_pool(name="lstm_dram", bufs=2, space="DRAM") as ldram,
    ):
        h_last = None
        for l in range(KN_LAYERS):
            wx = wp.tile([P, 36, 256], f32r, tag="wx")
            wh = wp.tile([P, 36, 256], f32r, tag="wh")
            nc.sync.dma_start(wx[:], wx_l[l].ap().rearrange("(tk p) m -> p tk m", p=P))
            nc.sync.dma_start(wh[:], wh_l[l].ap().rearrange("(tk p) m -> p tk m", p=P))
            bias = wp.tile([P, 2, 1], f32, tag="lbias")
            nc.sync.dma_start(bias[:], lb_l[l].ap().rearrange("(ch p) o -> p ch o", p=P))

            xp = plp.tile([P, 4, B, 18, 18], f32r, tag="xp")
            hp = plp.tile([P, 4, B, 18, 18], f32r, tag="hp")
            zero_fill(xp[:].rearrange("p a b c d -> p (a b c d)"))
            zero_fill(hp[:].rearrange("p a b c d -> p (a b c d)"))
            gx = gxp.tile([P, 2, KN_STEPS, NPX], f32, tag="gx")

            for t in range(KN_STEPS):
                for k in range(4):
                    for b in range(B):
                        if l == 0:
                            src = gath_feat[b * 4 + t, k * P:(k + 1) * P, :]
                        else:
                            src = gath_h[l - 1][t][k * P:(k + 1) * P, b, :]
                        nc.sync.dma_start(xp[:, k, b, 1:17, 1:17],
                                          src.rearrange("c (y x) -> c y x", y=16))
                for m in range(2):
                    for nh in range(2):
                        pt = ps.tile([P, 512], f32, tag="lps")
                        first = True
                        for tap in range(9):
                            ky, kx = tap // 3, tap % 3
                            for k in range(4):
                                nc.tensor.matmul(
                                    pt[:], wx[:, tap * 4 + k, m * P:(m + 1) * P],
                                    xp[:, k, nh * 2:nh * 2 + 2, ky:ky + 16, kx:kx + 16],
                                    start=first, stop=(tap == 8 and k == 3))
                                first = False
                        nc.scalar.activation(gx[:, m, t, nh * 512:(nh + 1) * 512], pt[:],
                                             AF.Identity, bias=bias[:, m])

            c_t = sb.tile([P, NPX], f32, tag="c_t")
            for t in range(KN_STEPS):
                if t > 0:
                    for k in range(4):
                        for b in range(B):
                            src = gath_h[l][t - 1][k * P:(k + 1) * P, b, :]
                            nc.sync.dma_start(hp[:, k, b, 1:17, 1:17],
                                              src.rearrange("c (y x) -> c y x", y=16))
                    sAB = []
                    for m in range(2):
                        sm = sb.tile([P, NPX], f32, tag=f"s{m}")
                        for nh in range(2):
                            pt = ps.tile([P, 512], f32, tag="lps")
                            first = True
                            for tap in range(9):
                                ky, kx = tap // 3, tap % 3
                                for k in range(4):
                                    nc.tensor.matmul(
                                        pt[:], wh[:, tap * 4 + k, m * P:(m + 1) * P],
                                        hp[:, k, nh * 2:nh * 2 + 2, ky:ky + 16, kx:kx + 16],
                                        start=first, stop=(tap == 8 and k == 3))
                                    first = False
                            nc.vector.tensor_add(sm[:, nh * 512:(nh + 1) * 512], pt[:],
                                                 gx[:, m, t, nh * 512:(nh + 1) * 512])
                        sAB.append(sm)
                    sA, sB_ = sAB
                else:
                    sA = gx[:, 0, 0]
                    sB_ = gx[:, 1, 0]
                nc.scalar.activation(sA[:], sA[:], AF.Sigmoid)        # [sig(i)|sig(f)]
                nc.scalar.activation(sB_[0:64], sB_[0:64], AF.Tanh)    # tanh(g)
                nc.scalar.activation(sB_[64:128], sB_[64:128], AF.Sigmoid)  # sig(o)
                it = sb.tile([64, NPX], f32, tag="it")
                nc.vector.tensor_mul(it[:], sA[0:64], sB_[0:64])
                it_hi = sb.tile([P, NPX], f32, tag="ithi")
                nc.sync.dma_start(it_hi[64:128], it[:])
                if t > 0:
                    nc.vector.tensor_mul(c_t[64:128], sA[64:128], c_t[64:128])
                    nc.vector.tensor_add(c_t[64:128], c_t[64:128], it_hi[64:128])
                else:
                    nc.vector.tensor_copy(c_t[64:128], it_hi[64:128])
                tct = sb.tile([P, NPX], f32, tag="tct")
                nc.scalar.activation(tct[64:128], c_t[64:128], AF.Tanh)
                h_t = sb.tile([P, NPX], f32, tag="h_t")
                nc.vector.tensor_mul(h_t[64:128], sB_[64:128], tct[64:128])
                cc_in = ldram.tile([64, B, 256], f32r, tag="ccin")
                nc.sync.dma_start(cc_in[:],
                                  h_t[64:128].rearrange("c (b px) -> c b px", b=B).bitcast(f32r))
                nc.gpsimd.collective_compute(
                    "AllGather", OP.bypass, replica_groups=[list(range(NCORES))],
                    ins=[cc_in[:].opt()], outs=[gath_h[l][t][:].opt()])
                if l == KN_LAYERS - 1 and t == KN_STEPS - 1:
                    h_last = h_t
        if dbg_h is not None:
            nc.sync.dma_start(dbg_h.ap(), gath_h[KN_LAYERS - 1][KN_STEPS - 1][:].bitcast(f32))

        # AllToAll out_me slab distribution
        send = sb.tile([64, 8, 12 * 16], f32, tag="send")
        nc.vector.memset(send[:], 0.0)
        hl = h_last[64:128].rearrange("c (b y x) -> c b y x", b=B, y=16)
        for d in range(8):
            s_d, cp = d // 2, d % 2
            z0, z1 = (2, 12) if cp == 0 else (0, 10)
            img0 = 8 * cp - 2 + z0
            nc.sync.dma_start(
                send[:, d, z0 * 16:z1 * 16], hl[:, s_d, img0:img0 + (z1 - z0), :])
        a2a_send = ldram.tile([8, 64, 12, 16], f32r, tag="a2asend")
        nc.sync.dma_start(a2a_send[:].rearrange("d c z x -> c d (z x)"),
                          send[:].bitcast(f32r))
        nc.gpsimd.collective_compute(
            "AllToAll", OP.bypass, replica_groups=[list(range(NCORES))],
            ins=[a2a_send[:].opt()], outs=[a2a_recv[:].opt()])


def build_generators(nc, tc, a2a_recv, wg1, bg1, wg2, bg2, wg3, bg3, wg4, bg4,
                     wfin, bfin, wfuse, bfuse, wpad, yflag, topmask,
                     frame_out, flow_out, pred_out, zero128, zero_fill):
    with (
        tc.tile_pool(name="gen_w", bufs=1) as wp,
        tc.tile_pool(name="gen_w1", bufs=2) as wp1,
        tc.tile_pool(name="gen_sb", bufs=1) as sb,
        tc.tile_pool(name="gen_pl", bufs=1) as plp,
        tc.tile_pool(name="gen_ps", bufs=4, space="PSUM") as ps,
        tc.tile_pool(name="gen_dram", bufs=1, space="DRAM") as gdram,
    ):
        # TRUE-interleaved images in DRAM (+1 col pad to avoid AP merging).
        def ibuf(nm, cch, rr, cc):
            return [gdram.tile([cch, rr, cc + 1], f32r, tag=f"{nm}_{g}", name=f"{nm}_{g}")
                    for g in range(2)]
        s1 = ibuf("s1", 512, 32, 32)
        s2 = ibuf("s2", 256, 36, 64)
        s3 = ibuf("s3", 128, 68, 128)
        s4 = ibuf("s4", 64, 132, 256)
        warp_dram = gdram.tile([3, 128, 256], f32r, tag="warp_dram")

        def store_phase(dstbuf, cs0, csz, py, px, rt0, rn, win, stg):
            # per-row DMAs: dst [c, win step-2 cols], src [c, win]
            for r in range(rn):
                nc.sync.dma_start(
                    dstbuf[cs0:cs0 + csz, 2 * (rt0 + r) + py, px:2 * win - 1 + px:2],
                    stg[:csz, r, :win].bitcast(f32r))

        # ---- L1 merged (M = 1024) ----
        zt = plp.tile([P, 4, 17, 17], f32r, tag="zt")
        zero_fill(zt[:].rearrange("p a b c -> p (a b c)"))
        for k in range(4):
            for half in range(2):
                nc.sync.dma_start(zt[half * 64:half * 64 + 64, k, 0:12, 0:16],
                                  a2a_recv[k * 2 + half])
        b1b = wp.tile([P, 8, 1], f32, tag="bias1")
        nc.sync.dma_start(b1b[:], bg1.ap().rearrange("(mm p) o -> p mm o", p=P))
        for py in (0, 1):
            for px in (0, 1):
                taps = [(ky, kx, dy, dx) for (a_, b_, ky, kx, dy, dx) in DEC_TAPS
                        if a_ == py and b_ == px]
                for mg in range(2):
                    pts = [ps.tile([P, 16, 16], f32, tag="gps", name=f"l1p_{py}{px}{mg}{_i}")
                           for _i in range(4)]
                    for ti, (ky, kx, dy, dx) in enumerate(taps):
                        tap_idx = DEC_TAPS.index((py, px, ky, kx, dy, dx))
                        wt = wp1.tile([P, 4, 512], f32r, tag="wg1t")
                        nc.sync.dma_start(
                            wt[:],
                            wg1.ap().rearrange("(tk p) m -> p tk m", p=P)[
                                :, tap_idx * 4:(tap_idx + 1) * 4, mg * 512:(mg + 1) * 512])
                        for k in range(4):
                            for mi in range(4):
                                nc.tensor.matmul(
                                    pts[mi][:].rearrange("c y x -> c (y x)"),
                                    wt[:, k, mi * P:(mi + 1) * P],
                                    zt[:, k, dy:dy + 16, dx:dx + 16],
                                    start=(ti == 0 and k == 0),
                                    stop=(ti == len(taps) - 1 and k == 3))
                    for mi in range(4):
                        m = mg * 4 + mi
                        g, mm_ = m // 4, m % 4
                        stg = sb.tile([P, 16, 17], f32, tag="l1stg")
                        nc.scalar.activation(stg[:, :, :16], pts[mi][:], AF.Relu,
                                             bias=b1b[:, m])
                        store_phase(s1[g], mm_ * P, P, py, px, 0, 16, 16, stg)

        # ---- L2..L4 per gen ----
        for (wd, bd, srcs, r0_, r1_, dsts, cin, win, mout) in (
                (wg2, bg2, s1, 3, 21, s2, 512, 32, 256),
                (wg3, bg3, s2, 1, 35, s3, 256, 64, 128),
                (wg4, bg4, s3, 1, 67, s4, 128, 128, 64)):
            kch = cin // P
            nrows = r1_ - r0_
            msize = min(P, mout)
            mchunks = mout // msize
            rn_max = max(1, 512 // win)
            for g in range(2):
                wt = wp.tile([P, 9 * kch, mout], f32r, tag=f"wg_{cin}")
                nc.sync.dma_start(wt[:], wd[g].ap().rearrange("(tk p) m -> p tk m", p=P))
                bt = wp.tile([msize, mchunks, 1], f32, tag=f"bg_{cin}")
                nc.sync.dma_start(bt[:], bd[g].ap().rearrange("(mm p) o -> p mm o", p=msize))
                xt = plp.tile([P, kch, nrows + 1, win + 1], f32r, tag=f"gpl_{cin}")
                zero_fill(xt[:].rearrange("p a b c -> p (a b c)"))
                for k in range(kch):
                    nc.sync.dma_start(xt[:, k, 0:nrows, 0:win],
                                      srcs[g][k * P:(k + 1) * P, r0_:r1_, :win])
                for py in (0, 1):
                    for px in (0, 1):
                        taps = [(ky, kx, dy, dx) for (a_, b_, ky, kx, dy, dx) in DEC_TAPS
                                if a_ == py and b_ == px]
                        for m in range(mchunks):
                            for rt0, rn in row_tiles(nrows, win, 512):
                                npx = rn * win
                                pt = ps.tile([P, rn_max, win], f32, tag="gps")
                                first = True
                                for ti, (ky, kx, dy, dx) in enumerate(taps):
                                    tap_idx = DEC_TAPS.index((py, px, ky, kx, dy, dx))
                                    for k in range(kch):
                                        nc.tensor.matmul(
                                            pt[:msize, :rn].rearrange("c y x -> c (y x)"),
                                            wt[:, tap_idx * kch + k, m * msize:(m + 1) * msize],
                                            xt[:, k, dy + rt0:dy + rt0 + rn, dx:dx + win],
                                            start=first,
                                            stop=(ti == len(taps) - 1 and k == kch - 1))
                                        first = False
                                stg = sb.tile([P, rn_max, win + 1], f32, tag="gstg")
                                nc.scalar.activation(stg[:msize, :rn, :win],
                                                     pt[:msize, :rn], AF.Relu, bias=bt[:, m])
                                store_phase(dsts[g], m * msize, msize, py, px,
                                            rt0, rn, win, stg)

        # zero image row -1 on top-slab cores (final-conv zero padding)
        for g in range(2):
            mrow = sb.tile([64, 256], f32r, tag="maskrow")
            mmask = sb.tile([64, 256], f32, tag="maskval")
            nc.sync.dma_start(mrow[:], s4[g][:, 1, :256])
            nc.sync.dma_start(mmask[:], topmask.ap().bitcast(f32))
            nc.vector.tensor_mul(mrow[:].bitcast(f32), mrow[:].bitcast(f32), mmask[:])
            nc.sync.dma_start(s4[g][:, 1, :256], mrow[:])

        # ---- final conv (frame||flow K-stacked, M=5) ----
        wfin_t = wp.tile([P, 9, 5], f32r, tag="wfin")
        nc.sync.dma_start(wfin_t[:], wfin.ap().rearrange("(t p) m -> p t m", p=P))
        bfin_t = wp.tile([5, 1], f32, tag="bfin")
        nc.sync.dma_start(bfin_t[:], bfin.ap())
        fin_pl = plp.tile([P, 12, 258], f32r, tag="fin_pl")
        zero_fill(fin_pl[:].rearrange("p a b -> p (a b)"))
        for rt0 in range(0, 128, 10):
            rn = min(10, 128 - rt0)
            for g in range(2):
                nc.sync.dma_start(fin_pl[g * 64:g * 64 + 64, 0:rn + 2, 1:257],
                                  s4[g][:, 1 + rt0:1 + rt0 + rn + 2, :256])
            for st0, sn in row_tiles(rn, 256, 512):
                npx = sn * 256
                pt = ps.tile([P, 512], f32, tag="gps")
                first = True
                for tap in range(9):
                    ky, kx = tap // 3, tap % 3
                    nc.tensor.matmul(pt[:5, :npx], wfin_t[:, tap, :],
                                     fin_pl[:, st0 + ky:st0 + ky + sn, kx:kx + 256],
                                     start=first, stop=(tap == 8))
                    first = False
                sig = sb.tile([5, 512], f32, tag="finsig")
                nc.scalar.activation(sig[:, :npx], pt[:5, :npx], AF.Sigmoid, bias=bfin_t[:])
                tnh = sb.tile([5, 512], f32, tag="fintanh")
                nc.scalar.activation(tnh[:, :npx], pt[:5, :npx], AF.Tanh, bias=bfin_t[:])
                rr = rt0 + st0
                nc.sync.dma_start(frame_out.ap()[:, rr:rr + sn, :],
                                  sig[0:3, :npx].rearrange("c (y x) -> c y x", y=sn))
                nc.sync.dma_start(flow_out.ap()[:, rr:rr + sn, :],
                                  tnh[3:5, :npx].rearrange("c (y x) -> c y x", y=sn))

        build_warp(nc, sb, wp, wpad, yflag, flow_out, warp_dram)

        # fuse 1x1 conv + sigmoid
        wfu = wp.tile([6, 3], f32r, tag="wfu")
        nc.sync.dma_start(wfu[:], wfuse.ap())
        bfu = wp.tile([3, 1], f32, tag="bfu")
        nc.sync.dma_start(bfu[:], bfuse.ap())
        frame_flat = frame_out.ap().rearrange("c y x -> c (y x)")
        warp_flat = warp_dram[:].rearrange("c y x -> c (y x)")
        for n0 in range(0, 128 * 256, 512):
            ft = sb.tile([6, 512], f32r, tag="fusein")
            nc.sync.dma_start(ft[0:3, :], frame_flat[:, n0:n0 + 512].bitcast(f32r))
            nc.sync.dma_start(ft[3:6, :], warp_flat[:, n0:n0 + 512])
            pt = ps.tile([P, 512], f32, tag="gps")
            nc.tensor.matmul(pt[:3], wfu[:], ft[:], start=True, stop=True)
            ot = sb.tile([3, 512], f32, tag="fuseo")
            nc.scalar.activation(ot[:], pt[:3], AF.Sigmoid, bias=bfu[:])
            nc.sync.dma_start(pred_out.ap().rearrange("c y x -> c (y x)")[:, n0:n0 + 512], ot[:])


def build_warp(nc, sb, wp, wpad, yflag, flow_out, warp_dram):
    yfl = wp.tile([1, 1], f32, tag="yfl")
    nc.sync.dma_start(yfl[:], yflag.ap())
    for chunk in range(2):
        r0 = chunk * 64
        fx = sb.tile([64, 256], f32, tag="wfx")
        fy = sb.tile([64, 256], f32, tag="wfy")
        nc.sync.dma_start(fx[:], flow_out.ap()[0, r0:r0 + 64, :])
        nc.sync.dma_start(fy[:], flow_out.ap()[1, r0:r0 + 64, :])
        planes = {}
        for nm, f_ in (("x", fx), ("y", fy)):
            pp = sb.tile([64, 256], f32, tag=f"w{nm}p")
            mm_ = sb.tile([64, 256], f32, tag=f"w{nm}m")
            zz = sb.tile([64, 256], f32, tag=f"w{nm}0")
            nc.scalar.activation(pp[:], f_[:], AF.Relu)
            nc.scalar.activation(mm_[:], f_[:], AF.Relu, scale=-1.0)
            nc.scalar.activation(zz[:], f_[:], AF.Abs)
            nc.vector.tensor_scalar(zz[:], zz[:], -1.0, 1.0, OP.mult, OP.add)
            planes[nm] = (pp, mm_, zz)
        wxp, wxm, wx0 = planes["x"]
        wyp, wym, wy0 = planes["y"]
        # col-0 / row-0 correction factors
        tcol = sb.tile([64, 1], f32, tag="tcol")
        mneg = sb.tile([64, 1], f32, tag="mneg")
        nc.vector.tensor_scalar(mneg[:], fx[:, 0:1], 0.0, None, OP.is_lt)
        nc.vector.tensor_scalar(tcol[:], fx[:, 0:1], 1.0, None, OP.add)
        nc.vector.tensor_mul(tcol[:], tcol[:], mneg[:])
        trow = sb.tile([1, 256], f32, tag="trow")
        mrow = sb.tile([1, 256], f32, tag="mrow")
        nc.vector.tensor_scalar(mrow[:], fy[0:1, :], 0.0, None, OP.is_lt)
        nc.vector.tensor_scalar(trow[:], fy[0:1, :], 1.0, None, OP.add)
        nc.vector.tensor_mul(trow[:], trow[:], mrow[:])
        nc.vector.tensor_scalar_mul(trow[:], trow[:], yfl[:1])

        for ch in range(3):
            wrow = sb.tile([66, 258], f32, tag="wrow")
            nc.sync.dma_start(wrow[:], wpad.ap()[ch, r0:r0 + 66, :])
            shifted = []
            for si, nm in ((0, "im"), (1, "i0"), (2, "ip")):
                tl = sb.tile([64, 258], f32, tag=nm)
                nc.sync.dma_start(tl[:], wrow[si:si + 64, :])
                shifted.append(tl)
            X = []
            for img, nm in zip(shifted, ("xm", "x0", "xp")):
                xi = sb.tile([64, 256], f32, tag=f"X{nm}")
                tmp = sb.tile([64, 256], f32, tag=f"Xt{nm}")
                nc.vector.tensor_mul(xi[:], wx0[:], img[:, 1:257])
                nc.vector.tensor_mul(tmp[:], wxp[:], img[:, 2:258])
                nc.vector.tensor_add(xi[:], xi[:], tmp[:])
                nc.vector.tensor_mul(tmp[:], wxm[:], img[:, 0:256])
                nc.vector.tensor_add(xi[:], xi[:], tmp[:])
                d01 = sb.tile([64, 1], f32, tag=f"d{nm}")
                nc.vector.tensor_tensor(d01[:], img[:, 2:3], img[:, 1:2], OP.subtract)
                nc.vector.tensor_mul(d01[:], d01[:], tcol[:])
                nc.vector.tensor_add(xi[:, 0:1], xi[:, 0:1], d01[:])
                X.append(xi)
            Xm, X0, Xp = X
            v = sb.tile([64, 256], f32, tag="wv")
            tmp2 = sb.tile([64, 256], f32, tag="wvt")
            nc.vector.tensor_mul(v[:], wy0[:], X0[:])
            nc.vector.tensor_mul(tmp2[:], wyp[:], Xp[:])
            nc.vector.tensor_add(v[:], v[:], tmp2[:])
            nc.vector.tensor_mul(tmp2[:], wym[:], Xm[:])
            nc.vector.tensor_add(v[:], v[:], tmp2[:])
            if chunk == 0:
                dr = sb.tile([1, 256], f32, tag="dr")
                nc.vector.tensor_tensor(dr[:], Xp[0:1, :], X0[0:1, :], OP.subtract)
                nc.vector.tensor_mul(dr[:], dr[:], trow[:])
                nc.vector.tensor_add(v[0:1, :], v[0:1, :], dr[:])
            nc.sync.dma_start(warp_dram[ch, r0:r0 + 64, :], v[:].bitcast(f32r))


# ==================================================================== host

_CACHE = {}


def _prep_weights(cnn_ws, lstm_ws, lstm_bs, gframe_ws, gframe_bs, gflow_ws,
                  gflow_bs, fuse_w, fuse_b):
    f = np.float32
    out = {}
    w1 = np.asarray(cnn_ws[0], f)          # (64, 3, 4, 4)
    out["w1"] = np.ascontiguousarray(w1.transpose(2, 3, 1, 0).reshape(48, 64))
    for li, nm in ((1, "w2"), (2, "w3"), (3, "w4")):
        w = np.asarray(cnn_ws[li], f)      # (Co, Ci, 4, 4)
        Co, Ci = w.shape[:2]
        shifts = []
        for qy in range(2):
            for qx in range(2):
                blocks = []
                for ry in range(2):
                    for rx in range(2):
                        blocks.append(w[:, :, 2 * qy + ry, 2 * qx + rx].T)  # (Ci, Co)
                shifts.append(np.concatenate(blocks, 0))                    # (4Ci, Co)
        out[nm] = np.ascontiguousarray(np.stack(shifts, 0).reshape(4 * 4 * Ci // P * P, Co))

    # per-core LSTM slices are produced later (need core index)
    out["_lstm_w"] = [np.asarray(w, f) for w in lstm_ws]
    out["_lstm_b"] = [np.asarray(b, f) for b in lstm_bs]

    def gen_layer(w):                       # deconv (Co, Ci, 3, 3) -> [9*KCH*128, Co]
        w = np.asarray(w, f)
        Co, Ci = w.shape[:2]
        taps = []
        for (py, ky, dy) in DEC_YT:
            for (px, kx, dx) in DEC_YT:
                taps.append(w[:, :, ky, kx].T)   # (Ci, Co)
        return np.ascontiguousarray(np.stack(taps, 0).reshape(9 * Ci, Co))

    wf, wl = [np.asarray(w, f) for w in gframe_ws], [np.asarray(w, f) for w in gflow_ws]
    bf = [np.asarray(b, f) for b in gframe_bs]
    bl = [np.asarray(b, f) for b in gflow_bs]
    out["wg1"] = np.concatenate([
        gen_layer(wf[0]).reshape(9, 512, 512), gen_layer(wl[0]).reshape(9, 512, 512)],
        axis=2).reshape(9 * 512, 1024)
    out["bg1"] = np.concatenate([bf[0], bl[0]])[:, None]
    for i, nm in ((1, "wg2"), (2, "wg3"), (3, "wg4")):
        out[f"{nm}_0"] = gen_layer(wf[i])
        out[f"{nm}_1"] = gen_layer(wl[i])
        out[f"b{nm[1:]}_0"] = bf[i][:, None]
        out[f"b{nm[1:]}_1"] = bl[i][:, None]
    # final conv: K = 64frame || 64flow stacked, M = 5
    wff, wlf = np.asarray(gframe_ws[4], f), np.asarray(gflow_ws[4], f)  # (3,64,3,3),(2,64,3,3)
    taps = []
    for ky in range(3):
        for kx in range(3):
            blk = np.zeros((128, 5), f)
            blk[0:64, 0:3] = wff[:, :, ky, kx].T
            blk[64:128, 3:5] = wlf[:, :, ky, kx].T
            taps.append(blk)
    out["wfin"] = np.concatenate(taps, 0)
    out["bfin"] = np.concatenate([np.asarray(gframe_bs[4], f),
                                  np.asarray(gflow_bs[4], f)])[:, None]
    out["wfuse"] = np.ascontiguousarray(np.asarray(fuse_w, f)[:, :, 0, 0].T)
    out["bfuse"] = np.asarray(fuse_b, f)[:, None]
    return out


def _lstm_core_slices(wl, bl, c):
    rows = []
    for gate in range(4):
        rows.extend(range(gate * 512 + 64 * c, gate * 512 + 64 * c + 64))
    # chunk order: [i|f] then [g|o]
    rows = np.array(rows[:128] + rows[128:], np.int64)
    ws = wl[rows]                     # (256, 1024, 3, 3)
    wx = ws[:, :512].transpose(2, 3, 1, 0).reshape(9, 512, 256).reshape(9 * 512, 256)
    whp = ws[:, 512:].transpose(2, 3, 1, 0).reshape(9, 512, 256).reshape(9 * 512, 256)
    bb = bl[rows][:, None]
    return (np.ascontiguousarray(wx), np.ascontiguousarray(whp),
            np.ascontiguousarray(bb))


def _im2col_conv1(frames):
    # frames: (2, 3, 256, 256) -> (2, 48, 128*128), tap-major rows (t*3+c)
    fpad = np.pad(frames, ((0, 0), (0, 0), (1, 1), (1, 1)))
    taps = []
    for dy in range(4):
        for dx in range(4):
            taps.append(fpad[:, :, dy:dy + 256:2, dx:dx + 256:2])
    arr = np.stack(taps, 1)  # (2, 16, 3, 128, 128)
    return np.ascontiguousarray(arr.reshape(2, 48, 128 * 128))


def build_in_maps(inp):
    x = np.asarray(inp["x"], np.float32)
    wd = _prep_weights(inp["cnn_ws"], inp["lstm_ws"], inp["lstm_bs"],
                       inp["gframe_ws"], inp["gframe_bs"], inp["gflow_ws"],
                       inp["gflow_bs"], inp["fuse_w"], inp["fuse_b"])
    frames = x.transpose(0, 2, 1, 3, 4).reshape(B * T, C, H, W)
    in_maps = []
    shared = {k: v for k, v in wd.items() if not k.startswith("_")}
    for cidx in range(NCORES):
        m = dict(shared)
        m["enc_in"] = _im2col_conv1(frames[2 * cidx:2 * cidx + 2])
        for l in range(3):
            wx, wh_, bb = _lstm_core_slices(wd["_lstm_w"][l], wd["_lstm_b"][l], cidx)
            m[f"wx{l}"], m[f"wh{l}"], m[f"lb{l}"] = wx, wh_, bb
        s, cp = cidx // 2, cidx % 2
        prev = x[s, :, -1]                         # (3, 256, 256)
        prow = np.pad(prev, ((0, 0), (1, 1), (1, 1)), mode="edge")  # (3,258,258)
        m["wpad"] = np.ascontiguousarray(prow[:, 128 * cp:128 * cp + 130, :])
        m["yflag"] = np.array([[1.0 if cp == 0 else 0.0]], np.float32)
        m["topmask"] = np.full((64, 256), 0.0 if cp == 0 else 1.0, np.float32)
        in_maps.append(m)
    return in_maps


def kernel(x, cnn_ws, lstm_ws, lstm_bs, gframe_ws, gframe_bs, gflow_ws,
           gflow_bs, fuse_w, fuse_b):
    x = np.asarray(x, np.float32)
    in_maps = build_in_maps(dict(x=x, cnn_ws=cnn_ws, lstm_ws=lstm_ws,
                                 lstm_bs=lstm_bs, gframe_ws=gframe_ws,
                                 gframe_bs=gframe_bs, gflow_ws=gflow_ws,
                                 gflow_bs=gflow_bs, fuse_w=fuse_w, fuse_b=fuse_b))

    if "nc" not in _CACHE:
        _CACHE["nc"] = build_program()
    nc = _CACHE["nc"]

    from concourse.bass_utils import run_bass_kernel_spmd
    res = run_bass_kernel_spmd(nc, in_maps, core_ids=list(range(NCORES)))
    results = res.results

    frame_pred = np.zeros((B, 3, H, W), np.float32)
    flow_pred = np.zeros((B, 2, H, W), np.float32)
    prediction = np.zeros((B, 3, H, W), np.float32)
    for cidx in range(NCORES):
        s, cp = cidx // 2, cidx % 2
        sl = slice(128 * cp, 128 * cp + 128)
        frame_pred[s, :, sl] = results[cidx]["frame_out"]
        flow_pred[s, :, sl] = results[cidx]["flow_out"]
        prediction[s, :, sl] = results[cidx]["pred_out"]
    if STAGE == "enc":
        return results[0]["dbg_feat"]
    if STAGE == "lstm":
        return results[0]["dbg_h"]
    return frame_pred, flow_pred, prediction


# revision 37
# speedup vs baseline: 97.6611x; 1.0453x over previous
"""DualMotionGAN forward on 8 Trainium2 NeuronCores (Bass/Tile, float32r matmuls).

Distribution: encoder data-parallel over the 16 frames (2 per core, stride-2
convs phase-decomposed into full-K tap matmuls); 3-layer ConvLSTM 8-way
gate-channel split (each core computes 64 channels of each gate for all
samples; h is AllGathered every step); an AllToAll then hands each core the
(sample, row-half) slab of out_me it needs, so the two deconv generators,
the bilinear warp (elementwise: flow=tanh in (-1,1) touches only the 3x3
neighborhood), and the fuse conv run spatially split with no further
communication. All per-core variation enters via host-sliced inputs --
the device program is identical on every core.
"""
import os
import numpy as np

import concourse.bass as bass
from concourse import bacc
import concourse.mybir as mybir
import concourse.tile as tile

f32r = mybir.dt.float32r
f32 = mybir.dt.float32
AF = mybir.ActivationFunctionType
OP = mybir.AluOpType
P = 128
NCORES = 8
B, C, T, H, W = 4, 3, 4, 256, 256

# deconv tap table: (phase, kernel index, input shift)
DEC_YT = [(0, 1, 0), (1, 0, 0), (1, 2, 1)]
DEC_TAPS = [(py, px, ky, kx, dy, dx)
            for (py, ky, dy) in DEC_YT for (px, kx, dx) in DEC_YT]

STAGE = os.environ.get("KSTAGE", "full")
KN_LAYERS = int(os.environ.get("KN_LAYERS", "3"))
KN_STEPS = int(os.environ.get("KN_STEPS", "4"))


def row_tiles(nrows, width, maxn=512):
    rpt = max(1, maxn // width)
    out = []
    r = 0
    while r < nrows:
        n = min(rpt, nrows - r)
        out.append((r, n))
        r += n
    return out


# =================================================================== device

def build_program():
    nc = bacc.Bacc()

    enc_in = nc.dram_tensor("enc_in", [2, 48, 128 * 128], f32r, kind="ExternalInput")
    w1 = nc.dram_tensor("w1", [48, 64], f32r, kind="ExternalInput")
    w2 = nc.dram_tensor("w2", [4 * 2 * P, 128], f32r, kind="ExternalInput")
    w3 = nc.dram_tensor("w3", [4 * 4 * P, 256], f32r, kind="ExternalInput")
    w4 = nc.dram_tensor("w4", [4 * 8 * P, 512], f32r, kind="ExternalInput")
    wx_l = [nc.dram_tensor(f"wx{l}", [9 * 4 * P, 256], f32r, kind="ExternalInput") for l in range(3)]
    wh_l = [nc.dram_tensor(f"wh{l}", [9 * 4 * P, 256], f32r, kind="ExternalInput") for l in range(3)]
    lb_l = [nc.dram_tensor(f"lb{l}", [2 * P, 1], f32, kind="ExternalInput") for l in range(3)]
    wg1 = nc.dram_tensor("wg1", [9 * 4 * P, 1024], f32r, kind="ExternalInput")
    bg1 = nc.dram_tensor("bg1", [8 * P, 1], f32, kind="ExternalInput")
    wg2 = [nc.dram_tensor(f"wg2_{g}", [9 * 4 * P, 256], f32r, kind="ExternalInput") for g in range(2)]
    bg2 = [nc.dram_tensor(f"bg2_{g}", [2 * P, 1], f32, kind="ExternalInput") for g in range(2)]
    wg3 = [nc.dram_tensor(f"wg3_{g}", [9 * 2 * P, 128], f32r, kind="ExternalInput") for g in range(2)]
    bg3 = [nc.dram_tensor(f"bg3_{g}", [P, 1], f32, kind="ExternalInput") for g in range(2)]
    wg4 = [nc.dram_tensor(f"wg4_{g}", [9 * P, 64], f32r, kind="ExternalInput") for g in range(2)]
    bg4 = [nc.dram_tensor(f"bg4_{g}", [64, 1], f32, kind="ExternalInput") for g in range(2)]
    wfin = nc.dram_tensor("wfin", [9 * P, 5], f32r, kind="ExternalInput")
    bfin = nc.dram_tensor("bfin", [5, 1], f32, kind="ExternalInput")
    wfuse = nc.dram_tensor("wfuse", [6, 3], f32r, kind="ExternalInput")
    bfuse = nc.dram_tensor("bfuse", [3, 1], f32, kind="ExternalInput")
    wpad = nc.dram_tensor("wpad", [3, 130, 258], f32, kind="ExternalInput")
    topmask = nc.dram_tensor("topmask", [64, 256], f32r, kind="ExternalInput")
    yflag = nc.dram_tensor("yflag", [1, 1], f32, kind="ExternalInput")

    frame_out = nc.dram_tensor("frame_out", [3, 128, 256], f32, kind="ExternalOutput")
    flow_out = nc.dram_tensor("flow_out", [2, 128, 256], f32, kind="ExternalOutput")
    pred_out = nc.dram_tensor("pred_out", [3, 128, 256], f32, kind="ExternalOutput")
    dbg_feat = (nc.dram_tensor("dbg_feat", [16, 512, 256], f32, kind="ExternalOutput")
                if STAGE == "enc" else None)
    dbg_h = (nc.dram_tensor("dbg_h", [512, B, 256], f32, kind="ExternalOutput")
             if STAGE == "lstm" else None)

    with tile.TileContext(nc) as tc:
        with (
            tc.tile_pool(name="persist_dram", bufs=1, space="DRAM") as pdram,
            tc.tile_pool(name="zp", bufs=1) as zpool,
        ):
            zero128 = zpool.tile([P, 1024], f32)
            nc.vector.memset(zero128[:], 0.0)
            zdram = pdram.tile([P, 1024], f32r, tag="zdram")
            nc.sync.dma_start(zdram[:], zero128[:].bitcast(f32r))

            def zero_fill(ap):
                flat = ap  # expects [128, N] contiguous view
                n = flat.shape[-1]
                for n0 in range(0, n, 1024):
                    w_ = min(1024, n - n0)
                    nc.sync.dma_start(flat[:, n0:n0 + w_], zdram[:, :w_])

            gath_feat = pdram.tile([16, 512, 256], f32r)
            gath_h = [[pdram.tile([512, B, 256], f32r, tag=f"gh{l}_{t}", name=f"gh{l}_{t}")
                       for t in range(T)] for l in range(3)]
            a2a_recv = pdram.tile([8, 64, 12, 16], f32r)

            build_encoder(nc, tc, enc_in, w1, w2, w3, w4, gath_feat, pdram, zero128)
            if STAGE == "enc":
                nc.sync.dma_start(dbg_feat.ap(), gath_feat[:].bitcast(f32))
            if STAGE != "enc":
                build_lstm(nc, tc, wx_l, wh_l, lb_l, gath_feat, gath_h, a2a_recv,
                           pdram, dbg_h, zero_fill)
            if STAGE in ("gen", "full"):
                build_generators(nc, tc, a2a_recv, wg1, bg1, wg2, bg2, wg3, bg3,
                                 wg4, bg4, wfin, bfin, wfuse, bfuse, wpad, yflag,
                                 topmask, frame_out, flow_out, pred_out, zero128, zero_fill)
            else:
                for t_ in (frame_out, flow_out, pred_out):
                    tv = t_.ap().rearrange("c y x -> c (y x)")
                    for n0 in range(0, 128 * 256, 1024):
                        nc.sync.dma_start(tv[:, n0:n0 + 1024], zero128[:t_.shape[0], :])
    nc.compile()
    return nc


def build_encoder(nc, tc, enc_in, w1, w2, w3, w4, gath_feat, pdram, zero128):
    with (
        tc.tile_pool(name="enc_w", bufs=1) as wp,
        tc.tile_pool(name="enc_w4", bufs=2) as wp4,
        tc.tile_pool(name="enc_sb", bufs=3) as sb,
        tc.tile_pool(name="enc_pl", bufs=1) as plp,
        tc.tile_pool(name="enc_pl4", bufs=2) as plp4,
        tc.tile_pool(name="enc_ps", bufs=4, space="PSUM") as ps,
        tc.tile_pool(name="enc_ps4", bufs=4, space="PSUM") as ps4,
        tc.tile_pool(name="enc_dram", bufs=1, space="DRAM") as edram,
    ):
        alpha = wp.tile([P, 1], f32, tag="alpha")
        nc.vector.memset(alpha[:], 0.2)
        e1 = [edram.tile([64, 130, 130], f32r, tag=f"e1_{j}", name=f"e1_{j}") for j in range(2)]
        e2 = [edram.tile([128, 66, 66], f32r, tag=f"e2_{j}", name=f"e2_{j}") for j in range(2)]
        e3 = [edram.tile([256, 34, 34], f32r, tag=f"e3_{j}", name=f"e3_{j}") for j in range(2)]
        enc_out = pdram.tile([2, 512, 256], f32r)
        for j in range(2):
            for buf, cch, hp_ in ((e1[j], 64, 130), (e2[j], 128, 66), (e3[j], 256, 34)):
                for cc0 in range(0, cch, P):
                    cn = min(P, cch - cc0)
                    z = zero128[:cn, :1]
                    nc.sync.dma_start(buf[cc0:cc0 + cn, 0, :].bitcast(f32), zero128[:cn, :hp_])
                    nc.sync.dma_start(buf[cc0:cc0 + cn, hp_ - 1, :].bitcast(f32), zero128[:cn, :hp_])
                    nc.sync.dma_start(buf[cc0:cc0 + cn, 1:hp_ - 1, 0].bitcast(f32), zero128[:cn, :hp_ - 2])
                    nc.sync.dma_start(buf[cc0:cc0 + cn, 1:hp_ - 1, hp_ - 1].bitcast(f32), zero128[:cn, :hp_ - 2])

        # conv1 (K=48 host-im2col)
        w1t = wp.tile([48, 64], f32r, tag="w1")
        nc.sync.dma_start(w1t[:], w1.ap())
        for j in range(2):
            imt = plp.tile([48, 128 * 128], f32r, tag="im2col")
            nc.sync.dma_start(imt[:], enc_in.ap()[j])
            for r0, nr in row_tiles(128, 128, 512):
                pt = ps.tile([64, 512], f32, tag="eps")
                nc.tensor.matmul(pt[:, :nr * 128], w1t[:], imt[:, r0 * 128:(r0 + nr) * 128],
                                 start=True, stop=True)
                ot = sb.tile([64, 512], f32, tag="c1o")
                nc.scalar.activation(ot[:, :nr * 128], pt[:, :nr * 128], AF.Prelu, alpha=alpha[:64])
                nc.sync.dma_start(e1[j][:, 1 + r0:1 + r0 + nr, 1:129],
                                  ot[:, :nr * 128].rearrange("c (y x) -> c y x", y=nr).bitcast(f32r))

        # conv2 / conv3 (shifts inner, weights fully resident)
        for li, (wdram, src, dst, cin, hin2, hout, mout) in enumerate((
                (w2, e1, e2, 64, 65, 64, 128),
                (w3, e2, e3, 128, 33, 32, 256))):
            kch = 4 * cin // P
            wt = wp.tile([P, 4 * kch, mout], f32r, tag=f"wenc{li}")
            nc.sync.dma_start(wt[:], wdram.ap().rearrange("(sk p) m -> p sk m", p=P))
            for j in range(2):
                xt = plp.tile([P, kch, hin2, hin2], f32r, tag=f"pl{li}")
                srcr = src[j][:].rearrange("c (i py) (j2 px) -> c py px i j2", py=2, px=2)
                for pl in range(4):
                    ry, rx = pl // 2, pl % 2
                    if cin == 64:
                        nc.sync.dma_start(xt[(pl % 2) * 64:(pl % 2) * 64 + 64, pl // 2],
                                          srcr[:, ry, rx])
                    else:
                        nc.sync.dma_start(xt[:, pl], srcr[:, ry, rx])
                for m in range(mout // P):
                    for r0, nr in row_tiles(hout, hout, 512):
                        npx = nr * hout
                        pt = ps.tile([P, 512], f32, tag="eps")
                        first = True
                        for s in range(4):
                            qy, qx = s // 2, s % 2
                            for k in range(kch):
                                nc.tensor.matmul(
                                    pt[:, :npx], wt[:, s * kch + k, m * P:(m + 1) * P],
                                    xt[:, k, qy + r0:qy + r0 + nr, qx:qx + hout],
                                    start=first, stop=(s == 3 and k == kch - 1))
                                first = False
                        ot = sb.tile([P, 512], f32, tag=f"c{li}o")
                        nc.scalar.activation(ot[:, :npx], pt[:, :npx], AF.Prelu, alpha=alpha[:])
                        nc.sync.dma_start(
                            dst[j][m * P:(m + 1) * P, 1 + r0:1 + r0 + nr, 1:1 + hout],
                            ot[:, :npx].rearrange("c (y x) -> c y x", y=nr).bitcast(f32r))

        # conv4: j outer, shifts outer (PSUM held across shifts), w4 loaded per shift
        for j in range(2):
            xt = plp4.tile([P, 8, 17, 17], f32r, tag="pl3")
            srcr = e3[j][:].rearrange("c (i py) (j2 px) -> c py px i j2", py=2, px=2)
            for pl in range(4):
                ry, rx = pl // 2, pl % 2
                for hh in range(2):
                    nc.sync.dma_start(xt[:, pl * 2 + hh], srcr[hh * P:(hh + 1) * P, ry, rx])
            pts = [ps4.tile([P, 256], f32, tag="e4ps", name=f"e4ps_{j}_{_i}") for _i in range(4)]
            for s in range(4):
                qy, qx = s // 2, s % 2
                wt4 = wp4.tile([P, 8, 512], f32r, tag="w4s")
                nc.sync.dma_start(
                    wt4[:], w4.ap().rearrange("(sk p) m -> p sk m", p=P)[:, s * 8:(s + 1) * 8, :])
                for m in range(4):
                    for k in range(8):
                        nc.tensor.matmul(
                            pts[m][:], wt4[:, k, m * P:(m + 1) * P],
                            xt[:, k, qy:qy + 16, qx:qx + 16],
                            start=(s == 0 and k == 0), stop=(s == 3 and k == 7))
            for m in range(4):
                ot = sb.tile([P, 256], f32, tag="c4o")
                nc.scalar.activation(ot[:], pts[m][:], AF.Prelu, alpha=alpha[:])
                nc.sync.dma_start(enc_out[j, m * P:(m + 1) * P, :], ot[:].bitcast(f32r))

        nc.gpsimd.collective_compute(
            "AllGather", OP.bypass, replica_groups=[list(range(NCORES))],
            ins=[enc_out[:].opt()], outs=[gath_feat[:].opt()])


def build_lstm(nc, tc, wx_l, wh_l, lb_l, gath_feat, gath_h, a2a_recv, pdram, dbg_h, zero_fill):
    NPX = B * 256
    with (
        tc.tile_pool(name="lstm_w", bufs=1) as wp,
        tc.tile_pool(name="lstm_sb", bufs=1) as sb,
        tc.tile_pool(name="lstm_gx", bufs=1) as gxp,
        tc.tile_pool(name="lstm_pl", bufs=1) as plp,
        tc.tile_pool(name="lstm_ps", bufs=4, space="PSUM") as ps,
        tc.tile_pool(name="lstm_dram", bufs=2, space="DRAM") as ldram,
    ):
        h_last = None
        for l in range(KN_LAYERS):
            wx = wp.tile([P, 36, 256], f32r, tag="wx")
            wh = wp.tile([P, 36, 256], f32r, tag="wh")
            nc.sync.dma_start(wx[:], wx_l[l].ap().rearrange("(tk p) m -> p tk m", p=P))
            nc.sync.dma_start(wh[:], wh_l[l].ap().rearrange("(tk p) m -> p tk m", p=P))
            bias = wp.tile([P, 2, 1], f32, tag="lbias")
            nc.sync.dma_start(bias[:], lb_l[l].ap().rearrange("(ch p) o -> p ch o", p=P))

            xps = [plp.tile([P, 4, B, 18, 18], f32r, tag=f"xp{i}", name=f"xp{i}") for i in range(2)]
            hp = plp.tile([P, 4, B, 18, 18], f32r, tag="hp")
            for tl in xps:
                zero_fill(tl[:].rearrange("p a b c d -> p (a b c d)"))
            zero_fill(hp[:].rearrange("p a b c d -> p (a b c d)"))
            gx = ldram.tile([P, 2, KN_STEPS, NPX], f32, tag="gx")

            for t in range(KN_STEPS):
                xp = xps[t % 2]
                for k in range(4):
                    for b in range(B):
                        if l == 0:
                            src = gath_feat[b * 4 + t, k * P:(k + 1) * P, :]
                        else:
                            src = gath_h[l - 1][t][k * P:(k + 1) * P, b, :]
                        nc.sync.dma_start(xp[:, k, b, 1:17, 1:17],
                                          src.rearrange("c (y x) -> c y x", y=16))
                for m in range(2):
                    for nh in range(2):
                        pt = ps.tile([P, 512], f32, tag="lps")
                        first = True
                        for tap in range(9):
                            ky, kx = tap // 3, tap % 3
                            for k in range(4):
                                nc.tensor.matmul(
                                    pt[:], wx[:, tap * 4 + k, m * P:(m + 1) * P],
                                    xp[:, k, nh * 2:nh * 2 + 2, ky:ky + 16, kx:kx + 16],
                                    start=first, stop=(tap == 8 and k == 3))
                                first = False
                        gxe = sb.tile([P, 512], f32, tag="gxe")
                        nc.scalar.activation(gxe[:], pt[:], AF.Identity, bias=bias[:, m])
                        nc.sync.dma_start(gx[:, m, t, nh * 512:(nh + 1) * 512], gxe[:])

            c_t = sb.tile([P, NPX], f32, tag="c_t")
            for t in range(KN_STEPS):
                if t > 0:
                    for k in range(4):
                        for b in range(B):
                            src = gath_h[l][t - 1][k * P:(k + 1) * P, b, :]
                            nc.sync.dma_start(hp[:, k, b, 1:17, 1:17],
                                              src.rearrange("c (y x) -> c y x", y=16))
                    sAB = []
                    for m in range(2):
                        sm = sb.tile([P, NPX], f32, tag=f"s{m}")
                        gxin = sb.tile([P, NPX], f32, tag="gxin")
                        nc.sync.dma_start(gxin[:], gx[:, m, t])
                        for nh in range(2):
                            pt = ps.tile([P, 512], f32, tag="lps")
                            first = True
                            for tap in range(9):
                                ky, kx = tap // 3, tap % 3
                                for k in range(4):
                                    nc.tensor.matmul(
                                        pt[:], wh[:, tap * 4 + k, m * P:(m + 1) * P],
                                        hp[:, k, nh * 2:nh * 2 + 2, ky:ky + 16, kx:kx + 16],
                                        start=first, stop=(tap == 8 and k == 3))
                                    first = False
                            nc.vector.tensor_add(sm[:, nh * 512:(nh + 1) * 512], pt[:],
                                                 gxin[:, nh * 512:(nh + 1) * 512])
                        sAB.append(sm)
                    sA, sB_ = sAB
                else:
                    sA = sb.tile([P, NPX], f32, tag="s0")
                    sB_ = sb.tile([P, NPX], f32, tag="s1")
                    nc.sync.dma_start(sA[:], gx[:, 0, 0])
                    nc.sync.dma_start(sB_[:], gx[:, 1, 0])
                nc.scalar.activation(sA[:], sA[:], AF.Sigmoid)        # [sig(i)|sig(f)]
                nc.scalar.activation(sB_[0:64], sB_[0:64], AF.Tanh)    # tanh(g)
                nc.scalar.activation(sB_[64:128], sB_[64:128], AF.Sigmoid)  # sig(o)
                it = sb.tile([64, NPX], f32, tag="it")
                nc.vector.tensor_mul(it[:], sA[0:64], sB_[0:64])
                it_hi = sb.tile([P, NPX], f32, tag="ithi")
                nc.sync.dma_start(it_hi[64:128], it[:])
                if t > 0:
                    nc.vector.tensor_mul(c_t[64:128], sA[64:128], c_t[64:128])
                    nc.vector.tensor_add(c_t[64:128], c_t[64:128], it_hi[64:128])
                else:
                    nc.vector.tensor_copy(c_t[64:128], it_hi[64:128])
                tct = sb.tile([P, NPX], f32, tag="tct")
                nc.scalar.activation(tct[64:128], c_t[64:128], AF.Tanh)
                h_t = sb.tile([P, NPX], f32, tag="h_t")
                nc.vector.tensor_mul(h_t[64:128], sB_[64:128], tct[64:128])
                cc_in = ldram.tile([64, B, 256], f32r, tag="ccin")
                nc.sync.dma_start(cc_in[:],
                                  h_t[64:128].rearrange("c (b px) -> c b px", b=B).bitcast(f32r))
                nc.gpsimd.collective_compute(
                    "AllGather", OP.bypass, replica_groups=[list(range(NCORES))],
                    ins=[cc_in[:].opt()], outs=[gath_h[l][t][:].opt()])
                if l == KN_LAYERS - 1 and t == KN_STEPS - 1:
                    h_last = h_t
        if dbg_h is not None:
            nc.sync.dma_start(dbg_h.ap(), gath_h[KN_LAYERS - 1][KN_STEPS - 1][:].bitcast(f32))

        # AllToAll out_me slab distribution
        send = sb.tile([64, 8, 12 * 16], f32, tag="send")
        nc.vector.memset(send[:], 0.0)
        hl = h_last[64:128].rearrange("c (b y x) -> c b y x", b=B, y=16)
        for d in range(8):
            s_d, cp = d // 2, d % 2
            z0, z1 = (2, 12) if cp == 0 else (0, 10)
            img0 = 8 * cp - 2 + z0
            nc.sync.dma_start(
                send[:, d, z0 * 16:z1 * 16], hl[:, s_d, img0:img0 + (z1 - z0), :])
        a2a_send = ldram.tile([8, 64, 12, 16], f32r, tag="a2asend")
        nc.sync.dma_start(a2a_send[:].rearrange("d c z x -> c d (z x)"),
                          send[:].bitcast(f32r))
        nc.gpsimd.collective_compute(
            "AllToAll", OP.bypass, replica_groups=[list(range(NCORES))],
            ins=[a2a_send[:].opt()], outs=[a2a_recv[:].opt()])


def build_generators(nc, tc, a2a_recv, wg1, bg1, wg2, bg2, wg3, bg3, wg4, bg4,
                     wfin, bfin, wfuse, bfuse, wpad, yflag, topmask,
                     frame_out, flow_out, pred_out, zero128, zero_fill):
    with (
        tc.tile_pool(name="gen_w", bufs=1) as wp,
        tc.tile_pool(name="gen_w1", bufs=2) as wp1,
        tc.tile_pool(name="gen_sb", bufs=1) as sb,
        tc.tile_pool(name="gen_pl", bufs=1) as plp,
        tc.tile_pool(name="gen_ps", bufs=4, space="PSUM") as ps,
        tc.tile_pool(name="gen_dram", bufs=1, space="DRAM") as gdram,
    ):
        # TRUE-interleaved images in DRAM (+1 col pad to avoid AP merging).
        def ibuf(nm, cch, rr, cc):
            return [gdram.tile([cch, rr, cc + 1], f32r, tag=f"{nm}_{g}", name=f"{nm}_{g}")
                    for g in range(2)]
        s1 = ibuf("s1", 512, 32, 32)
        s2 = ibuf("s2", 256, 36, 64)
        s3 = ibuf("s3", 128, 68, 128)
        s4 = ibuf("s4", 64, 132, 256)
        warp_dram = gdram.tile([3, 128, 256], f32r, tag="warp_dram")

        def store_phase(dstbuf, cs0, csz, py, px, rt0, rn, win, stg):
            # per-row DMAs: dst [c, win step-2 cols], src [c, win]
            for r in range(rn):
                nc.sync.dma_start(
                    dstbuf[cs0:cs0 + csz, 2 * (rt0 + r) + py, px:2 * win - 1 + px:2],
                    stg[:csz, r, :win].bitcast(f32r))

        # ---- L1 merged (M = 1024) ----
        zt = plp.tile([P, 4, 17, 17], f32r, tag="zt")
        zero_fill(zt[:].rearrange("p a b c -> p (a b c)"))
        for k in range(4):
            for half in range(2):
                nc.sync.dma_start(zt[half * 64:half * 64 + 64, k, 0:12, 0:16],
                                  a2a_recv[k * 2 + half])
        b1b = wp.tile([P, 8, 1], f32, tag="bias1")
        nc.sync.dma_start(b1b[:], bg1.ap().rearrange("(mm p) o -> p mm o", p=P))
        for py in (0, 1):
            for px in (0, 1):
                taps = [(ky, kx, dy, dx) for (a_, b_, ky, kx, dy, dx) in DEC_TAPS
                        if a_ == py and b_ == px]
                for mg in range(2):
                    pts = [ps.tile([P, 16, 16], f32, tag="gps", name=f"l1p_{py}{px}{mg}{_i}")
                           for _i in range(4)]
                    for ti, (ky, kx, dy, dx) in enumerate(taps):
                        tap_idx = DEC_TAPS.index((py, px, ky, kx, dy, dx))
                        wt = wp1.tile([P, 4, 512], f32r, tag="wg1t")
                        nc.sync.dma_start(
                            wt[:],
                            wg1.ap().rearrange("(tk p) m -> p tk m", p=P)[
                                :, tap_idx * 4:(tap_idx + 1) * 4, mg * 512:(mg + 1) * 512])
                        for k in range(4):
                            for mi in range(4):
                                nc.tensor.matmul(
                                    pts[mi][:].rearrange("c y x -> c (y x)"),
                                    wt[:, k, mi * P:(mi + 1) * P],
                                    zt[:, k, dy:dy + 16, dx:dx + 16],
                                    start=(ti == 0 and k == 0),
                                    stop=(ti == len(taps) - 1 and k == 3))
                    for mi in range(4):
                        m = mg * 4 + mi
                        g, mm_ = m // 4, m % 4
                        stg = sb.tile([P, 16, 17], f32, tag="l1stg")
                        nc.scalar.activation(stg[:, :, :16], pts[mi][:], AF.Relu,
                                             bias=b1b[:, m])
                        store_phase(s1[g], mm_ * P, P, py, px, 0, 16, 16, stg)

        # ---- L2..L4 per gen ----
        for (wd, bd, srcs, r0_, r1_, dsts, cin, win, mout) in (
                (wg2, bg2, s1, 3, 21, s2, 512, 32, 256),
                (wg3, bg3, s2, 1, 35, s3, 256, 64, 128),
                (wg4, bg4, s3, 1, 67, s4, 128, 128, 64)):
            kch = cin // P
            nrows = r1_ - r0_
            msize = min(P, mout)
            mchunks = mout // msize
            rn_max = max(1, 512 // win)
            for g in range(2):
                wt = wp.tile([P, 9 * kch, mout], f32r, tag=f"wg_{cin}")
                nc.sync.dma_start(wt[:], wd[g].ap().rearrange("(tk p) m -> p tk m", p=P))
                bt = wp.tile([msize, mchunks, 1], f32, tag=f"bg_{cin}")
                nc.sync.dma_start(bt[:], bd[g].ap().rearrange("(mm p) o -> p mm o", p=msize))
                xt = plp.tile([P, kch, nrows + 1, win + 1], f32r, tag=f"gpl_{cin}")
                zero_fill(xt[:].rearrange("p a b c -> p (a b c)"))
                for k in range(kch):
                    nc.sync.dma_start(xt[:, k, 0:nrows, 0:win],
                                      srcs[g][k * P:(k + 1) * P, r0_:r1_, :win])
                for py in (0, 1):
                    for px in (0, 1):
                        taps = [(ky, kx, dy, dx) for (a_, b_, ky, kx, dy, dx) in DEC_TAPS
                                if a_ == py and b_ == px]
                        for m in range(mchunks):
                            for rt0, rn in row_tiles(nrows, win, 512):
                                npx = rn * win
                                pt = ps.tile([P, rn_max, win], f32, tag="gps")
                                first = True
                                for ti, (ky, kx, dy, dx) in enumerate(taps):
                                    tap_idx = DEC_TAPS.index((py, px, ky, kx, dy, dx))
                                    for k in range(kch):
                                        nc.tensor.matmul(
                                            pt[:msize, :rn].rearrange("c y x -> c (y x)"),
                                            wt[:, tap_idx * kch + k, m * msize:(m + 1) * msize],
                                            xt[:, k, dy + rt0:dy + rt0 + rn, dx:dx + win],
                                            start=first,
                                            stop=(ti == len(taps) - 1 and k == kch - 1))
                                        first = False
                                stg = sb.tile([P, rn_max, win + 1], f32, tag="gstg")
                                nc.scalar.activation(stg[:msize, :rn, :win],
                                                     pt[:msize, :rn], AF.Relu, bias=bt[:, m])
                                store_phase(dsts[g], m * msize, msize, py, px,
                                            rt0, rn, win, stg)

        # zero image row -1 on top-slab cores (final-conv zero padding)
        for g in range(2):
            mrow = sb.tile([64, 256], f32r, tag="maskrow")
            mmask = sb.tile([64, 256], f32, tag="maskval")
            nc.sync.dma_start(mrow[:], s4[g][:, 1, :256])
            nc.sync.dma_start(mmask[:], topmask.ap().bitcast(f32))
            nc.vector.tensor_mul(mrow[:].bitcast(f32), mrow[:].bitcast(f32), mmask[:])
            nc.sync.dma_start(s4[g][:, 1, :256], mrow[:])

        # ---- final conv (frame||flow K-stacked, M=5) ----
        wfin_t = wp.tile([P, 9, 5], f32r, tag="wfin")
        nc.sync.dma_start(wfin_t[:], wfin.ap().rearrange("(t p) m -> p t m", p=P))
        bfin_t = wp.tile([5, 1], f32, tag="bfin")
        nc.sync.dma_start(bfin_t[:], bfin.ap())
        fin_pl = plp.tile([P, 12, 258], f32r, tag="fin_pl")
        zero_fill(fin_pl[:].rearrange("p a b -> p (a b)"))
        for rt0 in range(0, 128, 10):
            rn = min(10, 128 - rt0)
            for g in range(2):
                nc.sync.dma_start(fin_pl[g * 64:g * 64 + 64, 0:rn + 2, 1:257],
                                  s4[g][:, 1 + rt0:1 + rt0 + rn + 2, :256])
            for st0, sn in row_tiles(rn, 256, 512):
                npx = sn * 256
                pt = ps.tile([P, 512], f32, tag="gps")
                first = True
                for tap in range(9):
                    ky, kx = tap // 3, tap % 3
                    nc.tensor.matmul(pt[:5, :npx], wfin_t[:, tap, :],
                                     fin_pl[:, st0 + ky:st0 + ky + sn, kx:kx + 256],
                                     start=first, stop=(tap == 8))
                    first = False
                sig = sb.tile([5, 512], f32, tag="finsig")
                nc.scalar.activation(sig[:, :npx], pt[:5, :npx], AF.Sigmoid, bias=bfin_t[:])
                tnh = sb.tile([5, 512], f32, tag="fintanh")
                nc.scalar.activation(tnh[:, :npx], pt[:5, :npx], AF.Tanh, bias=bfin_t[:])
                rr = rt0 + st0
                nc.sync.dma_start(frame_out.ap()[:, rr:rr + sn, :],
                                  sig[0:3, :npx].rearrange("c (y x) -> c y x", y=sn))
                nc.sync.dma_start(flow_out.ap()[:, rr:rr + sn, :],
                                  tnh[3:5, :npx].rearrange("c (y x) -> c y x", y=sn))

        build_warp(nc, sb, wp, wpad, yflag, flow_out, warp_dram)

        # fuse 1x1 conv + sigmoid
        wfu = wp.tile([6, 3], f32r, tag="wfu")
        nc.sync.dma_start(wfu[:], wfuse.ap())
        bfu = wp.tile([3, 1], f32, tag="bfu")
        nc.sync.dma_start(bfu[:], bfuse.ap())
        frame_flat = frame_out.ap().rearrange("c y x -> c (y x)")
        warp_flat = warp_dram[:].rearrange("c y x -> c (y x)")
        for n0 in range(0, 128 * 256, 512):
            ft = sb.tile([6, 512], f32r, tag="fusein")
            nc.sync.dma_start(ft[0:3, :], frame_flat[:, n0:n0 + 512].bitcast(f32r))
            nc.sync.dma_start(ft[3:6, :], warp_flat[:, n0:n0 + 512])
            pt = ps.tile([P, 512], f32, tag="gps")
            nc.tensor.matmul(pt[:3], wfu[:], ft[:], start=True, stop=True)
            ot = sb.tile([3, 512], f32, tag="fuseo")
            nc.scalar.activation(ot[:], pt[:3], AF.Sigmoid, bias=bfu[:])
            nc.sync.dma_start(pred_out.ap().rearrange("c y x -> c (y x)")[:, n0:n0 + 512], ot[:])


def build_warp(nc, sb, wp, wpad, yflag, flow_out, warp_dram):
    yfl = wp.tile([1, 1], f32, tag="yfl")
    nc.sync.dma_start(yfl[:], yflag.ap())
    for chunk in range(2):
        r0 = chunk * 64
        fx = sb.tile([64, 256], f32, tag="wfx")
        fy = sb.tile([64, 256], f32, tag="wfy")
        nc.sync.dma_start(fx[:], flow_out.ap()[0, r0:r0 + 64, :])
        nc.sync.dma_start(fy[:], flow_out.ap()[1, r0:r0 + 64, :])
        planes = {}
        for nm, f_ in (("x", fx), ("y", fy)):
            pp = sb.tile([64, 256], f32, tag=f"w{nm}p")
            mm_ = sb.tile([64, 256], f32, tag=f"w{nm}m")
            zz = sb.tile([64, 256], f32, tag=f"w{nm}0")
            nc.scalar.activation(pp[:], f_[:], AF.Relu)
            nc.scalar.activation(mm_[:], f_[:], AF.Relu, scale=-1.0)
            nc.scalar.activation(zz[:], f_[:], AF.Abs)
            nc.vector.tensor_scalar(zz[:], zz[:], -1.0, 1.0, OP.mult, OP.add)
            planes[nm] = (pp, mm_, zz)
        wxp, wxm, wx0 = planes["x"]
        wyp, wym, wy0 = planes["y"]
        # col-0 / row-0 correction factors
        tcol = sb.tile([64, 1], f32, tag="tcol")
        mneg = sb.tile([64, 1], f32, tag="mneg")
        nc.vector.tensor_scalar(mneg[:], fx[:, 0:1], 0.0, None, OP.is_lt)
        nc.vector.tensor_scalar(tcol[:], fx[:, 0:1], 1.0, None, OP.add)
        nc.vector.tensor_mul(tcol[:], tcol[:], mneg[:])
        trow = sb.tile([1, 256], f32, tag="trow")
        mrow = sb.tile([1, 256], f32, tag="mrow")
        nc.vector.tensor_scalar(mrow[:], fy[0:1, :], 0.0, None, OP.is_lt)
        nc.vector.tensor_scalar(trow[:], fy[0:1, :], 1.0, None, OP.add)
        nc.vector.tensor_mul(trow[:], trow[:], mrow[:])
        nc.vector.tensor_scalar_mul(trow[:], trow[:], yfl[:1])

        for ch in range(3):
            wrow = sb.tile([66, 258], f32, tag="wrow")
            nc.sync.dma_start(wrow[:], wpad.ap()[ch, r0:r0 + 66, :])
            shifted = []
            for si, nm in ((0, "im"), (1, "i0"), (2, "ip")):
                tl = sb.tile([64, 258], f32, tag=nm)
                nc.sync.dma_start(tl[:], wrow[si:si + 64, :])
                shifted.append(tl)
            X = []
            for img, nm in zip(shifted, ("xm", "x0", "xp")):
                xi = sb.tile([64, 256], f32, tag=f"X{nm}")
                tmp = sb.tile([64, 256], f32, tag=f"Xt{nm}")
                nc.vector.tensor_mul(xi[:], wx0[:], img[:, 1:257])
                nc.vector.tensor_mul(tmp[:], wxp[:], img[:, 2:258])
                nc.vector.tensor_add(xi[:], xi[:], tmp[:])
                nc.vector.tensor_mul(tmp[:], wxm[:], img[:, 0:256])
                nc.vector.tensor_add(xi[:], xi[:], tmp[:])
                d01 = sb.tile([64, 1], f32, tag=f"d{nm}")
                nc.vector.tensor_tensor(d01[:], img[:, 2:3], img[:, 1:2], OP.subtract)
                nc.vector.tensor_mul(d01[:], d01[:], tcol[:])
                nc.vector.tensor_add(xi[:, 0:1], xi[:, 0:1], d01[:])
                X.append(xi)
            Xm, X0, Xp = X
            v = sb.tile([64, 256], f32, tag="wv")
            tmp2 = sb.tile([64, 256], f32, tag="wvt")
            nc.vector.tensor_mul(v[:], wy0[:], X0[:])
            nc.vector.tensor_mul(tmp2[:], wyp[:], Xp[:])
            nc.vector.tensor_add(v[:], v[:], tmp2[:])
            nc.vector.tensor_mul(tmp2[:], wym[:], Xm[:])
            nc.vector.tensor_add(v[:], v[:], tmp2[:])
            if chunk == 0:
                dr = sb.tile([1, 256], f32, tag="dr")
                nc.vector.tensor_tensor(dr[:], Xp[0:1, :], X0[0:1, :], OP.subtract)
                nc.vector.tensor_mul(dr[:], dr[:], trow[:])
                nc.vector.tensor_add(v[0:1, :], v[0:1, :], dr[:])
            nc.sync.dma_start(warp_dram[ch, r0:r0 + 64, :], v[:].bitcast(f32r))


# ==================================================================== host

_CACHE = {}


def _prep_weights(cnn_ws, lstm_ws, lstm_bs, gframe_ws, gframe_bs, gflow_ws,
                  gflow_bs, fuse_w, fuse_b):
    f = np.float32
    out = {}
    w1 = np.asarray(cnn_ws[0], f)          # (64, 3, 4, 4)
    out["w1"] = np.ascontiguousarray(w1.transpose(2, 3, 1, 0).reshape(48, 64))
    for li, nm in ((1, "w2"), (2, "w3"), (3, "w4")):
        w = np.asarray(cnn_ws[li], f)      # (Co, Ci, 4, 4)
        Co, Ci = w.shape[:2]
        shifts = []
        for qy in range(2):
            for qx in range(2):
                blocks = []
                for ry in range(2):
                    for rx in range(2):
                        blocks.append(w[:, :, 2 * qy + ry, 2 * qx + rx].T)  # (Ci, Co)
                shifts.append(np.concatenate(blocks, 0))                    # (4Ci, Co)
        out[nm] = np.ascontiguousarray(np.stack(shifts, 0).reshape(4 * 4 * Ci // P * P, Co))

    # per-core LSTM slices are produced later (need core index)
    out["_lstm_w"] = [np.asarray(w, f) for w in lstm_ws]
    out["_lstm_b"] = [np.asarray(b, f) for b in lstm_bs]

    def gen_layer(w):                       # deconv (Co, Ci, 3, 3) -> [9*KCH*128, Co]
        w = np.asarray(w, f)
        Co, Ci = w.shape[:2]
        taps = []
        for (py, ky, dy) in DEC_YT:
            for (px, kx, dx) in DEC_YT:
                taps.append(w[:, :, ky, kx].T)   # (Ci, Co)
        return np.ascontiguousarray(np.stack(taps, 0).reshape(9 * Ci, Co))

    wf, wl = [np.asarray(w, f) for w in gframe_ws], [np.asarray(w, f) for w in gflow_ws]
    bf = [np.asarray(b, f) for b in gframe_bs]
    bl = [np.asarray(b, f) for b in gflow_bs]
    out["wg1"] = np.concatenate([
        gen_layer(wf[0]).reshape(9, 512, 512), gen_layer(wl[0]).reshape(9, 512, 512)],
        axis=2).reshape(9 * 512, 1024)
    out["bg1"] = np.concatenate([bf[0], bl[0]])[:, None]
    for i, nm in ((1, "wg2"), (2, "wg3"), (3, "wg4")):
        out[f"{nm}_0"] = gen_layer(wf[i])
        out[f"{nm}_1"] = gen_layer(wl[i])
        out[f"b{nm[1:]}_0"] = bf[i][:, None]
        out[f"b{nm[1:]}_1"] = bl[i][:, None]
    # final conv: K = 64frame || 64flow stacked, M = 5
    wff, wlf = np.asarray(gframe_ws[4], f), np.asarray(gflow_ws[4], f)  # (3,64,3,3),(2,64,3,3)
    taps = []
    for ky in range(3):
        for kx in range(3):
            blk = np.zeros((128, 5), f)
            blk[0:64, 0:3] = wff[:, :, ky, kx].T
            blk[64:128, 3:5] = wlf[:, :, ky, kx].T
            taps.append(blk)
    out["wfin"] = np.concatenate(taps, 0)
    out["bfin"] = np.concatenate([np.asarray(gframe_bs[4], f),
                                  np.asarray(gflow_bs[4], f)])[:, None]
    out["wfuse"] = np.ascontiguousarray(np.asarray(fuse_w, f)[:, :, 0, 0].T)
    out["bfuse"] = np.asarray(fuse_b, f)[:, None]
    return out


def _lstm_core_slices(wl, bl, c):
    rows = []
    for gate in range(4):
        rows.extend(range(gate * 512 + 64 * c, gate * 512 + 64 * c + 64))
    # chunk order: [i|f] then [g|o]
    rows = np.array(rows[:128] + rows[128:], np.int64)
    ws = wl[rows]                     # (256, 1024, 3, 3)
    wx = ws[:, :512].transpose(2, 3, 1, 0).reshape(9, 512, 256).reshape(9 * 512, 256)
    whp = ws[:, 512:].transpose(2, 3, 1, 0).reshape(9, 512, 256).reshape(9 * 512, 256)
    bb = bl[rows][:, None]
    return (np.ascontiguousarray(wx), np.ascontiguousarray(whp),
            np.ascontiguousarray(bb))


def _im2col_conv1(frames):
    # frames: (2, 3, 256, 256) -> (2, 48, 128*128), tap-major rows (t*3+c)
    fpad = np.pad(frames, ((0, 0), (0, 0), (1, 1), (1, 1)))
    taps = []
    for dy in range(4):
        for dx in range(4):
            taps.append(fpad[:, :, dy:dy + 256:2, dx:dx + 256:2])
    arr = np.stack(taps, 1)  # (2, 16, 3, 128, 128)
    return np.ascontiguousarray(arr.reshape(2, 48, 128 * 128))


def build_in_maps(inp):
    x = np.asarray(inp["x"], np.float32)
    wd = _prep_weights(inp["cnn_ws"], inp["lstm_ws"], inp["lstm_bs"],
                       inp["gframe_ws"], inp["gframe_bs"], inp["gflow_ws"],
                       inp["gflow_bs"], inp["fuse_w"], inp["fuse_b"])
    frames = x.transpose(0, 2, 1, 3, 4).reshape(B * T, C, H, W)
    in_maps = []
    shared = {k: v for k, v in wd.items() if not k.startswith("_")}
    for cidx in range(NCORES):
        m = dict(shared)
        m["enc_in"] = _im2col_conv1(frames[2 * cidx:2 * cidx + 2])
        for l in range(3):
            wx, wh_, bb = _lstm_core_slices(wd["_lstm_w"][l], wd["_lstm_b"][l], cidx)
            m[f"wx{l}"], m[f"wh{l}"], m[f"lb{l}"] = wx, wh_, bb
        s, cp = cidx // 2, cidx % 2
        prev = x[s, :, -1]                         # (3, 256, 256)
        prow = np.pad(prev, ((0, 0), (1, 1), (1, 1)), mode="edge")  # (3,258,258)
        m["wpad"] = np.ascontiguousarray(prow[:, 128 * cp:128 * cp + 130, :])
        m["yflag"] = np.array([[1.0 if cp == 0 else 0.0]], np.float32)
        m["topmask"] = np.full((64, 256), 0.0 if cp == 0 else 1.0, np.float32)
        in_maps.append(m)
    return in_maps


def kernel(x, cnn_ws, lstm_ws, lstm_bs, gframe_ws, gframe_bs, gflow_ws,
           gflow_bs, fuse_w, fuse_b):
    x = np.asarray(x, np.float32)
    in_maps = build_in_maps(dict(x=x, cnn_ws=cnn_ws, lstm_ws=lstm_ws,
                                 lstm_bs=lstm_bs, gframe_ws=gframe_ws,
                                 gframe_bs=gframe_bs, gflow_ws=gflow_ws,
                                 gflow_bs=gflow_bs, fuse_w=fuse_w, fuse_b=fuse_b))

    if "nc" not in _CACHE:
        _CACHE["nc"] = build_program()
    nc = _CACHE["nc"]

    from concourse.bass_utils import run_bass_kernel_spmd
    res = run_bass_kernel_spmd(nc, in_maps, core_ids=list(range(NCORES)))
    results = res.results

    frame_pred = np.zeros((B, 3, H, W), np.float32)
    flow_pred = np.zeros((B, 2, H, W), np.float32)
    prediction = np.zeros((B, 3, H, W), np.float32)
    for cidx in range(NCORES):
        s, cp = cidx // 2, cidx % 2
        sl = slice(128 * cp, 128 * cp + 128)
        frame_pred[s, :, sl] = results[cidx]["frame_out"]
        flow_pred[s, :, sl] = results[cidx]["flow_out"]
        prediction[s, :, sl] = results[cidx]["pred_out"]
    if STAGE == "enc":
        return results[0]["dbg_feat"]
    if STAGE == "lstm":
        return results[0]["dbg_h"]
    return frame_pred, flow_pred, prediction
